# revision 10
# baseline (speedup 1.0000x reference)
"""DualPrimalEdgePooling on 8 TRN2 NeuronCores.

Strategy (graph/data parallel with halo exchange, per the sharding hint):
  - Host computes the O(E) int32 bookkeeping: top-k pool mask, connected
    components (union-find by min label), cluster compaction, dual-pair
    compaction, and the gather/packing tables that drive the device.
  - The 8-core SPMD Bass kernel does all O(N*F)/O(E*F) feature work:
      * primal_x sharded by rows across cores; each core column-sum reduces
        its shard on the TensorEngine (for the giant component's mean).
      * each core indirect-DMA gathers the small-cluster / dual-segment
        member rows that live in its shard, packed by destination core.
      * one AllToAll (halo exchange) routes member rows to the core that
        owns their output segment.
      * per 128-segment group: one-hot (iota==slot) selection matrix matmul
        accumulates segment sums in PSUM; mean via per-partition reciprocal
        scale; direct DMA to compact per-core outputs.
      * new_primal_edge_index is rebuilt on-device (is_equal + select),
        cluster passes through the device.
  - Host scatters the compact device outputs into the full-size (mostly
    zero) result tensors.
"""

import os
import sys

for _p in ("/opt/trn_rl_repo",):
    if _p not in sys.path and os.path.isdir(_p):
        sys.path.insert(0, _p)

import numpy as np

import concourse.bass as bass
import concourse.bacc as bacc
import concourse.mybir as mybir
from concourse.bass_utils import run_bass_kernel_spmd
from concourse.tile import TileContext

N = 200_000
E = 600_000
F = 128
NUM_KEEP = 300_000
W = 8                     # cores
NS = N // W               # primal rows per shard  (25000)
ES = E // W               # dual rows / edges per shard (75000)
NSP = ((NS + 127) // 128) * 128   # padded shard rows (25088)
ESP = ((ES + 127) // 128) * 128   # padded shard rows (75008)
OOB = 1 << 30             # out-of-bounds gather index (skipped)

_last_results = None      # BassKernelResults of the most recent run (for test harness)


# --------------------------------------------------------------------------
# host-side graph bookkeeping
# --------------------------------------------------------------------------

def _pool_mask(att):
    order = np.argsort(-att, kind="stable")
    m = np.zeros(E, dtype=bool)
    m[order[: E - NUM_KEEP]] = True
    return m


def _connected_components(src, dst, mask):
    """labels[i] = min node id in i's component over masked edges."""
    parent = np.arange(N, dtype=np.int64)

    def find(a):
        while parent[a] != a:
            parent[a] = parent[parent[a]]
            a = parent[a]
        return a

    for a, b in zip(src[mask], dst[mask]):
        ra, rb = find(a), find(b)
        if ra != rb:
            if ra < rb:
                parent[rb] = ra
            else:
                parent[ra] = rb
    # final root resolution (roots are component minima by union-by-min)
    lab = np.empty(N, dtype=np.int64)
    for i in range(N):
        lab[i] = find(i)
    return lab


def _group_members(keys, num_keys):
    """For int array keys (>=0), return list-of-arrays members per key value."""
    order = np.argsort(keys, kind="stable")
    sk = keys[order]
    starts = np.searchsorted(sk, np.arange(num_keys), side="left")
    ends = np.searchsorted(sk, np.arange(num_keys), side="right")
    return [order[s:e] for s, e in zip(starts, ends)]


class _Side:
    """Packing tables for one segment-sum side (primal clusters / dual segs)."""

    def __init__(self, item_ids, members_per_item, shard_size):
        self.item_ids = item_ids
        # ownership: round-robin over item rank — balanced and decorrelated
        # from the member-id/shard correlation of sorted cluster ids
        M = len(item_ids)
        owner = np.arange(M) % W
        self.owned = [np.nonzero(owner == k)[0] for k in range(W)]  # idx into item_ids
        self.G = max(1, max((len(o) + 127) // 128 for o in self.owned))
        # canonical member walk per receiver core -> send lists + member entries
        self.sends = [[[] for _ in range(W)] for _ in range(W)]  # [sender][receiver] -> local row ids
        entries = [[] for _ in range(W)]  # per receiver: (sender, pos_in_send, slot_in_group, group)
        for k in range(W):
            for gi, ii in enumerate(self.owned[k]):
                g = gi // 128
                slot = gi % 128
                for gid in members_per_item[ii]:
                    j = gid // shard_size
                    lid = gid % shard_size
                    self.sends[j][k].append(lid)
                    entries[k].append((j, len(self.sends[j][k]) - 1, slot, g))
        self.entries = entries
        smax = max(len(self.sends[j][k]) for j in range(W) for k in range(W))
        self.S = ((smax + 127) // 128) * 128 if smax else 128
        # member tiles per group (uniform across groups/cores)
        tm = 1
        for k in range(W):
            cnt = np.zeros(self.G, dtype=np.int64)
            for (_, _, _, g) in entries[k]:
                cnt[g] += 1
            if len(entries[k]):
                tm = max(tm, int(np.max((cnt + 127) // 128)))
        self.Tm = tm

    def finalize(self, region_off, R):
        """Build per-core device arrays once R (recv row pitch) is known."""
        self.gidx = []   # [W] arrays [G*Tm*128] int32 recv-row indices
        self.slot = []   # [W] arrays [G*Tm*128] f32 slot ids
        for k in range(W):
            gi = np.full(self.G * self.Tm * 128, OOB, dtype=np.int64)
            sl = np.zeros(self.G * self.Tm * 128, dtype=np.float32)
            fill = np.zeros(self.G, dtype=np.int64)  # rows used per group
            for (j, p, slot, g) in self.entries[k]:
                pos = g * self.Tm * 128 + fill[g]
                gi[pos] = j * R + region_off + p
                sl[pos] = slot
                fill[g] += 1
            self.gidx.append(gi.astype(np.int32))
            self.slot.append(sl)

    def send_idx(self, j):
        """[W*S] int32 local gather rows for sender j, grouped by receiver."""
        out = np.full(W * self.S, OOB, dtype=np.int64)
        for k in range(W):
            rows = self.sends[j][k]
            out[k * self.S: k * self.S + len(rows)] = rows
        return out.astype(np.int32)

    def recips(self, k, counts_per_item):
        r = np.ones(self.G * 128, dtype=np.float32)
        o = self.owned[k]
        r[: len(o)] = 1.0 / np.maximum(counts_per_item[o], 1.0)
        return r


def _wrap128(a):
    """[T*128] -> [128, T] transposed tile layout (column t = tile t)."""
    assert a.size % 128 == 0
    return np.ascontiguousarray(a.reshape(-1, 128).T)


# --------------------------------------------------------------------------
# device program
# --------------------------------------------------------------------------

def _build_program(Gp, Tmp, Gd, Tmd, Sp, Sd, R, recip_giant):
    f32, i32 = mybir.dt.float32, mybir.dt.int32
    nc = bacc.Bacc(None, target_bir_lowering=False)

    px = nc.declare_dram_parameter("px", [NSP, F], f32, isOutput=False)
    dx = nc.declare_dram_parameter("dx", [ESP, F], f32, isOutput=False)
    p_sidx = nc.declare_dram_parameter("p_sidx", [128, W * Sp // 128], i32, isOutput=False)
    d_sidx = nc.declare_dram_parameter("d_sidx", [128, W * Sd // 128], i32, isOutput=False)
    p_gidx = nc.declare_dram_parameter("p_gidx", [128, Gp * Tmp], i32, isOutput=False)
    p_slot = nc.declare_dram_parameter("p_slot", [128, Gp * Tmp], f32, isOutput=False)
    d_gidx = nc.declare_dram_parameter("d_gidx", [128, Gd * Tmd], i32, isOutput=False)
    d_slot = nc.declare_dram_parameter("d_slot", [128, Gd * Tmd], f32, isOutput=False)
    p_recip = nc.declare_dram_parameter("p_recip", [128, Gp], f32, isOutput=False)
    d_recip = nc.declare_dram_parameter("d_recip", [128, Gd], f32, isOutput=False)
    cu_in = nc.declare_dram_parameter("cu", [128, ESP // 128], i32, isOutput=False)
    cv_in = nc.declare_dram_parameter("cv", [128, ESP // 128], i32, isOutput=False)
    cl_in = nc.declare_dram_parameter("cl", [128, NSP // 128], i32, isOutput=False)
    iota_in = nc.declare_dram_parameter("iota", [128, 128], f32, isOutput=False)

    out_p = nc.declare_dram_parameter("out_p", [Gp * 128, F], f32, isOutput=True)
    out_d = nc.declare_dram_parameter("out_d", [Gd * 128, F], f32, isOutput=True)
    out_e = nc.declare_dram_parameter("out_e", [2, 128, ESP // 128], i32, isOutput=True)
    out_cl = nc.declare_dram_parameter("out_cl", [128, NSP // 128], i32, isOutput=True)
    out_g = nc.declare_dram_parameter("out_g", [1, F], f32, isOutput=True)

    send_buf = nc.dram_tensor("send_buf", [W * R, F], f32)
    recv_buf = nc.dram_tensor("recv_buf", [W * R, F], f32)

    ET = ESP // 128  # edge tiles (columns in wrapped layout)
    NT = NSP // 128
    stats_off = Sp + Sd

    with TileContext(nc) as tc:
        with (
            tc.tile_pool(name="const", bufs=1) as cpool,
            tc.tile_pool(name="work", bufs=6) as wpool,
            tc.tile_pool(name="aux", bufs=2) as apool,
            tc.tile_pool(name="psa", bufs=1, space="PSUM") as psa,
            tc.tile_pool(name="psg", bufs=3, space="PSUM") as psg,
        )            :
            # ---- constants / preloaded tables ----
            ones = cpool.tile([128, 1], f32, tag="ones")
            nc.vector.memset(ones[:], 1.0)
            iota_f = cpool.tile([128, 128], f32, tag="iota")
            nc.sync.dma_start(out=iota_f[:], in_=iota_in[:, :])

            def preload(t, tag):
                s = cpool.tile(list(t.shape), t.dtype, tag=tag)
                nc.sync.dma_start(out=s[:], in_=t[:, :])
                return s

            p_sidx_s = preload(p_sidx, "p_sidx")
            d_sidx_s = preload(d_sidx, "d_sidx")
            p_gidx_s = preload(p_gidx, "p_gidx")
            p_slot_s = preload(p_slot, "p_slot")
            d_gidx_s = preload(d_gidx, "d_gidx")
            d_slot_s = preload(d_slot, "d_slot")
            p_recip_s = preload(p_recip, "p_recip")
            d_recip_s = preload(d_recip, "d_recip")

            # ---- edge index rebuild + cluster passthrough ----
            cu_t = apool.tile([128, ET], i32, tag="cu")
            cv_t = apool.tile([128, ET], i32, tag="cv")
            nc.sync.dma_start(out=cu_t[:], in_=cu_in[:, :])
            nc.sync.dma_start(out=cv_t[:], in_=cv_in[:, :])
            eq = apool.tile([128, ET], i32, tag="eq")
            nc.vector.tensor_tensor(out=eq[:], in0=cu_t[:], in1=cv_t[:],
                                    op=mybir.AluOpType.is_equal)
            neg1 = apool.tile([128, ET], i32, tag="neg1")
            nc.vector.memset(neg1[:], -1)
            r0 = apool.tile([128, ET], i32, tag="r0")
            nc.vector.select(out=r0[:], mask=eq[:], on_true=neg1[:], on_false=cu_t[:])
            nc.sync.dma_start(out=out_e[0, :, :], in_=r0[:])
            r1 = apool.tile([128, ET], i32, tag="r1")
            nc.vector.select(out=r1[:], mask=eq[:], on_true=neg1[:], on_false=cv_t[:])
            nc.sync.dma_start(out=out_e[1, :, :], in_=r1[:])

            cl_t = apool.tile([128, NT], i32, tag="cl")
            nc.sync.dma_start(out=cl_t[:], in_=cl_in[:, :])
            nc.sync.dma_start(out=out_cl[:, :], in_=cl_t[:])

            # ---- shard column-sum (for giant cluster) ----
            pxv = px[:, :].rearrange("(b t p) f -> b p t f", p=128, t=4)
            nb = pxv.shape[0]
            ps_cs = psa.tile([1, 512], f32, tag="ps_cs")
            for b in range(nb):
                xt = wpool.tile([128, 512], f32, tag="cs_x")
                nc.sync.dma_start(out=xt[:], in_=pxv[b, :, :, :])
                nc.tensor.matmul(out=ps_cs[:], lhsT=ones[:], rhs=xt[:],
                                 start=(b == 0), stop=(b == nb - 1))

            # ---- send-side gathers (halo exchange staging) ----
            ps_sm = psa.tile([1, 128], f32, tag="ps_sm")
            n_pt = W * Sp // 128
            for i in range(n_pt):
                xt = wpool.tile([128, F], f32, tag="send_x")
                nc.vector.memset(xt[:], 0.0)
                nc.gpsimd.indirect_dma_start(
                    out=xt[:], out_offset=None, in_=px[:, :],
                    in_offset=bass.IndirectOffsetOnAxis(ap=p_sidx_s[:, i:i + 1], axis=0),
                    bounds_check=NSP - 1, oob_is_err=False)
                nc.tensor.matmul(out=ps_sm[:], lhsT=ones[:], rhs=xt[:],
                                 start=(i == 0), stop=(i == n_pt - 1))
                k, t = divmod(i, Sp // 128)
                nc.sync.dma_start(out=send_buf[k * R + t * 128: k * R + t * 128 + 128, :],
                                  in_=xt[:])
            n_dt = W * Sd // 128
            for i in range(n_dt):
                xt = wpool.tile([128, F], f32, tag="send_x")
                nc.vector.memset(xt[:], 0.0)
                nc.gpsimd.indirect_dma_start(
                    out=xt[:], out_offset=None, in_=dx[:, :],
                    in_offset=bass.IndirectOffsetOnAxis(ap=d_sidx_s[:, i:i + 1], axis=0),
                    bounds_check=ESP - 1, oob_is_err=False)
                k, t = divmod(i, Sd // 128)
                base = k * R + Sp + t * 128
                nc.sync.dma_start(out=send_buf[base: base + 128, :], in_=xt[:])

            # ---- per-shard stats (total colsum, small-member colsum) ----
            stat_tot = apool.tile([1, 128], f32, tag="stat_tot")
            nc.vector.tensor_copy(out=stat_tot[:], in_=ps_cs[0:1, 0:128])
            for blk in range(1, 4):
                nc.vector.tensor_add(out=stat_tot[:], in0=stat_tot[:],
                                     in1=ps_cs[0:1, blk * 128:(blk + 1) * 128])
            stat_sml = apool.tile([1, 128], f32, tag="stat_sml")
            nc.vector.tensor_copy(out=stat_sml[:], in_=ps_sm[0:1, :])
            for k in range(W):
                nc.sync.dma_start(out=send_buf[k * R + stats_off: k * R + stats_off + 1, :],
                                  in_=stat_tot[:])
                nc.sync.dma_start(out=send_buf[k * R + stats_off + 1: k * R + stats_off + 2, :],
                                  in_=stat_sml[:])

            # ---- halo exchange ----
            nc.gpsimd.collective_compute(
                "AllToAll", mybir.AluOpType.bypass,
                replica_groups=[list(range(W))],
                ins=[send_buf[:, :]], outs=[recv_buf[:, :]])

            # ---- segment-sum groups ----
            def side(G, Tm, gidx_s, slot_s, recip_s, out_t):
                for g in range(G):
                    ps = psg.tile([128, F], f32, tag="ps_g")
                    for m in range(Tm):
                        j = g * Tm + m
                        xt = wpool.tile([128, F], f32, tag="seg_x")
                        nc.vector.memset(xt[:], 0.0)
                        nc.gpsimd.indirect_dma_start(
                            out=xt[:], out_offset=None, in_=recv_buf[:, :],
                            in_offset=bass.IndirectOffsetOnAxis(ap=gidx_s[:, j:j + 1], axis=0),
                            bounds_check=W * R - 1, oob_is_err=False)
                        sm = wpool.tile([128, 128], f32, tag="sel")
                        nc.vector.tensor_tensor(
                            out=sm[:], in0=slot_s[:, j:j + 1].to_broadcast([128, 128]),
                            in1=iota_f[:], op=mybir.AluOpType.is_equal)
                        nc.tensor.matmul(out=ps[:], lhsT=sm[:], rhs=xt[:],
                                         start=(m == 0), stop=(m == Tm - 1))
                    o = wpool.tile([128, F], f32, tag="seg_o")
                    nc.vector.tensor_scalar_mul(o[:], ps[:], recip_s[:, g:g + 1])
                    nc.sync.dma_start(out=out_t[g * 128:(g + 1) * 128, :], in_=o[:])

            side(Gp, Tmp, p_gidx_s, p_slot_s, p_recip_s, out_p)
            side(Gd, Tmd, d_gidx_s, d_slot_s, d_recip_s, out_d)

            # ---- giant cluster row ----
            tots = apool.tile([8, 128], f32, tag="tots")
            smls = apool.tile([8, 128], f32, tag="smls")
            for r in range(W):
                nc.sync.dma_start(out=tots[r:r + 1, :],
                                  in_=recv_buf[r * R + stats_off: r * R + stats_off + 1, :])
                nc.sync.dma_start(out=smls[r:r + 1, :],
                                  in_=recv_buf[r * R + stats_off + 1: r * R + stats_off + 2, :])
            ones8 = cpool.tile([8, 1], f32, tag="ones8")
            nc.vector.memset(ones8[:], 1.0)
            pt = psa.tile([1, 128], f32, tag="pt")
            nc.tensor.matmul(out=pt[:], lhsT=ones8[:], rhs=tots[:], start=True, stop=True)
            ps2 = psa.tile([1, 128], f32, tag="ps2")
            nc.tensor.matmul(out=ps2[:], lhsT=ones8[:], rhs=smls[:], start=True, stop=True)
            gr2 = apool.tile([1, 128], f32, tag="gr2")
            nc.vector.tensor_copy(out=gr2[:], in_=ps2[0:1, :])
            gr = apool.tile([1, 128], f32, tag="gr")
            nc.vector.tensor_tensor(out=gr[:], in0=pt[0:1, :], in1=gr2[:],
                                    op=mybir.AluOpType.subtract)
            nc.vector.tensor_scalar_mul(gr[:], gr[:], float(recip_giant))
            nc.sync.dma_start(out=out_g[:, :], in_=gr[:])

    nc.compile()
    return nc


# --------------------------------------------------------------------------
# main entry
# --------------------------------------------------------------------------

def kernel(primal_x, dual_x, att, primal_edge_index):
    global _last_results
    primal_x = np.asarray(primal_x, dtype=np.float32)
    dual_x = np.asarray(dual_x, dtype=np.float32)
    att = np.asarray(att, dtype=np.float32)
    pei = np.asarray(primal_edge_index, dtype=np.int32)
    src = pei[0].astype(np.int64)
    dst = pei[1].astype(np.int64)

    # ---------------- host graph bookkeeping ----------------
    mask = _pool_mask(att)
    labels = _connected_components(src, dst, mask)
    uniq, cluster = np.unique(labels, return_inverse=True)
    cluster = cluster.astype(np.int64)
    C = len(uniq)
    counts = np.bincount(cluster, minlength=C).astype(np.float64)
    giant = int(np.argmax(counts))

    cu = cluster[src]
    cv = cluster[dst]
    valid = cu != cv
    va = np.minimum(cu, cv)[valid]
    vb = np.maximum(cu, cv)[valid]
    vidx = np.nonzero(valid)[0]
    vp = np.stack([va, vb], axis=1)
    upairs, dinv = np.unique(vp, axis=0, return_inverse=True)
    D = len(upairs)
    dcounts = np.bincount(dinv, minlength=D).astype(np.float64)

    # members
    p_items = np.array([c for c in range(C) if c != giant], dtype=np.int64)
    members_all = _group_members(cluster, C)
    p_members = [members_all[c] for c in p_items]
    d_items = np.arange(D, dtype=np.int64)
    d_groups = _group_members(dinv, D)
    d_members = [vidx[g] for g in d_groups]  # global edge ids

    sp = _Side(p_items, p_members, NS)
    sd = _Side(d_items, d_members, ES)
    Sp, Sd = sp.S, sd.S
    R = Sp + Sd + 2
    sp.finalize(0, R)
    sd.finalize(Sp, R)
    Gp, Tmp, Gd, Tmd = sp.G, sp.Tm, sd.G, sd.Tm

    p_counts_per_item = counts[p_items]
    d_counts_per_item = dcounts

    # ---------------- per-core input maps ----------------
    iota128 = np.broadcast_to(np.arange(128, dtype=np.float32), (128, 128)).copy()
    in_maps = []
    for k in range(W):
        pxs = np.zeros((NSP, F), dtype=np.float32)
        pxs[:NS] = primal_x[k * NS:(k + 1) * NS]
        dxs = np.zeros((ESP, F), dtype=np.float32)
        dxs[:ES] = dual_x[k * ES:(k + 1) * ES]
        cus = np.zeros(ESP, dtype=np.int32)
        cvs = np.zeros(ESP, dtype=np.int32)
        cus[:ES] = cu[k * ES:(k + 1) * ES]
        cvs[:ES] = cv[k * ES:(k + 1) * ES]
        cls = np.zeros(NSP, dtype=np.int32)
        cls[:NS] = cluster[k * NS:(k + 1) * NS]
        in_maps.append({
            "px": pxs, "dx": dxs,
            "p_sidx": _wrap128(sp.send_idx(k)),
            "d_sidx": _wrap128(sd.send_idx(k)),
            "p_gidx": _wrap128(sp.gidx[k]),
            "p_slot": _wrap128(sp.slot[k]),
            "d_gidx": _wrap128(sd.gidx[k]),
            "d_slot": _wrap128(sd.slot[k]),
            "p_recip": _wrap128(sp.recips(k, p_counts_per_item).astype(np.float32)),
            "d_recip": _wrap128(sd.recips(k, d_counts_per_item).astype(np.float32)),
            "cu": _wrap128(cus), "cv": _wrap128(cvs), "cl": _wrap128(cls),
            "iota": iota128,
        })

    # ---------------- build + run ----------------
    nc = _build_program(Gp, Tmp, Gd, Tmd, Sp, Sd, R,
                        1.0 / max(counts[giant], 1.0))
    res = run_bass_kernel_spmd(nc, in_maps, list(range(W)))
    _last_results = res

    # ---------------- host assembly ----------------
    new_primal_x = np.zeros((N, F), dtype=np.float32)
    new_dual_x = np.zeros((E, F), dtype=np.float32)
    for k in range(W):
        o = sp.owned[k]
        if len(o):
            new_primal_x[p_items[o]] = res.results[k]["out_p"][: len(o)]
        od = sd.owned[k]
        if len(od):
            new_dual_x[d_items[od]] = res.results[k]["out_d"][: len(od)]
    new_primal_x[giant] = res.results[0]["out_g"][0]

    npei = np.empty((2, E), dtype=np.int32)
    for k in range(W):
        oe = res.results[k]["out_e"]  # [2, 128, ET] wrapped
        npei[0, k * ES:(k + 1) * ES] = oe[0].T.reshape(-1)[:ES]
        npei[1, k * ES:(k + 1) * ES] = oe[1].T.reshape(-1)[:ES]

    cl_out = np.empty(N, dtype=np.int32)
    for k in range(W):
        cl_out[k * NS:(k + 1) * NS] = res.results[k]["out_cl"].T.reshape(-1)[:NS]

    return new_primal_x, new_dual_x, npei, cl_out


# revision 13
# speedup vs baseline: 2.4871x; 2.4871x over previous
"""DualPrimalEdgePooling on 8 TRN2 NeuronCores.

Strategy (graph/data parallel, collective-free):
  - Host computes the O(E) int32 bookkeeping: top-k pool mask, connected
    components (union-find by min label), cluster compaction, dual-pair
    compaction, and the gather/packing tables that drive the device.
  - The 8-core SPMD Bass kernel does all O(N*F)/O(E*F) feature work with
    fully independent cores (collectives on this part measure ~70-110us of
    fixed barrier cost, so ownership is arranged to need none):
      * primal_x/dual_x are visible to every core; segment ownership is
        round-robin.  Each core indirect-DMA gathers the member rows of its
        owned segments, reduces them with one-hot (iota==slot) selection
        matmuls accumulated in PSUM, applies the mean via a per-partition
        reciprocal scale, and writes compact outputs.
      * the giant component's sum is (total - sum of small members): each
        core column-sum reduces one contiguous 1/8 shard of primal_x (DVE
        accumulate + one ones-matmul) and the colsum of its gathered small
        members, emitting a [2,128] partial; the final (tot-small)/count for
        that single output row is folded on the host during unsharding.
      * new_primal_edge_index is rebuilt on-device (is_equal + select),
        cluster passes through the device.
  - Host scatters the compact device outputs into the full-size (mostly
    zero) result tensors.
"""

import os
import sys

for _p in ("/opt/trn_rl_repo",):
    if _p not in sys.path and os.path.isdir(_p):
        sys.path.insert(0, _p)

import numpy as np

import concourse.bass as bass
import concourse.bacc as bacc
import concourse.mybir as mybir
from concourse.bass_utils import run_bass_kernel_spmd
from concourse.tile import TileContext

N = 200_000
E = 600_000
F = 128
NUM_KEEP = 300_000
W = 8                     # cores
NS = N // W               # primal rows per shard  (25000)
ES = E // W               # dual rows / edges per shard (75000)
NSP = ((NS + 127) // 128) * 128   # padded shard rows (25088)
ESP = ((ES + 127) // 128) * 128   # padded shard rows (75008)
NP_FULL = NSP * W                 # padded full primal rows (200704)
EP_FULL = ESP * W                 # padded full dual rows (600064)
GATHER_BATCH = 1                  # offset columns per indirect DMA (multi-column
                                  # offsets gather garbage — verified on HW)

_last_results = None      # BassKernelResults of the most recent run (for test harness)


# --------------------------------------------------------------------------
# host-side graph bookkeeping
# --------------------------------------------------------------------------

def _pool_mask(att):
    order = np.argsort(-att, kind="stable")
    m = np.zeros(E, dtype=bool)
    m[order[: E - NUM_KEEP]] = True
    return m


def _connected_components(src, dst, mask):
    """labels[i] = min node id in i's component over masked edges."""
    parent = np.arange(N, dtype=np.int64)

    def find(a):
        while parent[a] != a:
            parent[a] = parent[parent[a]]
            a = parent[a]
        return a

    for a, b in zip(src[mask], dst[mask]):
        ra, rb = find(a), find(b)
        if ra != rb:
            if ra < rb:
                parent[rb] = ra
            else:
                parent[ra] = rb
    lab = np.empty(N, dtype=np.int64)
    for i in range(N):
        lab[i] = find(i)
    return lab


def _group_members(keys, num_keys):
    """For int array keys (>=0), return list-of-arrays members per key value."""
    order = np.argsort(keys, kind="stable")
    sk = keys[order]
    starts = np.searchsorted(sk, np.arange(num_keys), side="left")
    ends = np.searchsorted(sk, np.arange(num_keys), side="right")
    return [order[s:e] for s, e in zip(starts, ends)]


class _Side:
    """Packing tables for one segment-sum side (primal clusters / dual segs).

    Items are assigned round-robin to cores; each core's owned items are
    grouped 128 per PSUM group; member rows are packed 128 per gather tile,
    Tm tiles per group (uniform across cores for the SPMD program)."""

    def __init__(self, item_ids, members_per_item, pad_row):
        self.item_ids = item_ids
        M = len(item_ids)
        owner = np.arange(M) % W
        self.owned = [np.nonzero(owner == k)[0] for k in range(W)]
        self.G = max(1, max((len(o) + 127) // 128 for o in self.owned))
        tm = 1
        percore = []
        for k in range(W):
            cnt = np.zeros(self.G, dtype=np.int64)
            ent = []
            for gi, ii in enumerate(self.owned[k]):
                g, slot = divmod(gi, 128)
                for gid in members_per_item[ii]:
                    ent.append((g, slot, gid))
                cnt[g] += len(members_per_item[ii])
            percore.append(ent)
            tm = max(tm, int(np.max((cnt + 127) // 128)) if len(ent) else 1)
        self.Tm = tm
        self.gidx = []
        self.slot = []
        for k in range(W):
            gi = np.full(self.G * self.Tm * 128, pad_row, dtype=np.int64)
            sl = np.zeros(self.G * self.Tm * 128, dtype=np.float32)
            fill = np.zeros(self.G, dtype=np.int64)
            for (g, slot, gid) in percore[k]:
                pos = g * self.Tm * 128 + fill[g]
                gi[pos] = gid
                sl[pos] = slot
                fill[g] += 1
            self.gidx.append(gi.astype(np.int32))
            self.slot.append(sl)

    def recips(self, k, counts_per_item):
        r = np.ones(self.G * 128, dtype=np.float32)
        o = self.owned[k]
        r[: len(o)] = 1.0 / np.maximum(counts_per_item[o], 1.0)
        return r


def _wrap128(a):
    """[T*128] -> [128, T] transposed tile layout (column t = tile t)."""
    assert a.size % 128 == 0
    return np.ascontiguousarray(a.reshape(-1, 128).T)


# --------------------------------------------------------------------------
# device program
# --------------------------------------------------------------------------

def _build_program(Gp, Tmp, Gd, Tmd, batched):
    f32, i32 = mybir.dt.float32, mybir.dt.int32
    nc = bacc.Bacc(None, target_bir_lowering=False)

    pxf = nc.declare_dram_parameter("pxf", [NP_FULL, F], f32, isOutput=False)
    dxf = nc.declare_dram_parameter("dxf", [EP_FULL, F], f32, isOutput=False)
    pxs = nc.declare_dram_parameter("pxs", [NSP, F], f32, isOutput=False)
    p_gidx = nc.declare_dram_parameter("p_gidx", [128, Gp * Tmp], i32, isOutput=False)
    p_slot = nc.declare_dram_parameter("p_slot", [128, Gp * Tmp], f32, isOutput=False)
    d_gidx = nc.declare_dram_parameter("d_gidx", [128, Gd * Tmd], i32, isOutput=False)
    d_slot = nc.declare_dram_parameter("d_slot", [128, Gd * Tmd], f32, isOutput=False)
    p_recip = nc.declare_dram_parameter("p_recip", [128, Gp], f32, isOutput=False)
    d_recip = nc.declare_dram_parameter("d_recip", [128, Gd], f32, isOutput=False)
    cu_in = nc.declare_dram_parameter("cu", [128, ESP // 128], i32, isOutput=False)
    cv_in = nc.declare_dram_parameter("cv", [128, ESP // 128], i32, isOutput=False)
    cl_in = nc.declare_dram_parameter("cl", [128, NSP // 128], i32, isOutput=False)
    iota_in = nc.declare_dram_parameter("iota", [128, 128], f32, isOutput=False)

    out_p = nc.declare_dram_parameter("out_p", [Gp * 128, F], f32, isOutput=True)
    out_d = nc.declare_dram_parameter("out_d", [Gd * 128, F], f32, isOutput=True)
    out_e = nc.declare_dram_parameter("out_e", [2, 128, ESP // 128], i32, isOutput=True)
    out_cl = nc.declare_dram_parameter("out_cl", [128, NSP // 128], i32, isOutput=True)
    out_st = nc.declare_dram_parameter("out_st", [2, F], f32, isOutput=True)

    ET = ESP // 128
    NT = NSP // 128

    with TileContext(nc) as tc:
        with (
            tc.tile_pool(name="const", bufs=1) as cpool,
            tc.tile_pool(name="work", bufs=6) as wpool,
            tc.tile_pool(name="aux", bufs=1) as apool,
            tc.tile_pool(name="psa", bufs=1, space="PSUM") as psa,
            tc.tile_pool(name="psg", bufs=3, space="PSUM") as psg,
        ):
            # ---- constants / preloaded tables ----
            ones = cpool.tile([128, 1], f32, tag="ones")
            nc.vector.memset(ones[:], 1.0)
            iota_f = cpool.tile([128, 128], f32, tag="iota")
            nc.sync.dma_start(out=iota_f[:], in_=iota_in[:, :])

            def preload(t, tag):
                s = cpool.tile(list(t.shape), t.dtype, tag=tag)
                nc.sync.dma_start(out=s[:], in_=t[:, :])
                return s

            p_gidx_s = preload(p_gidx, "p_gidx")
            p_slot_s = preload(p_slot, "p_slot")
            d_gidx_s = preload(d_gidx, "d_gidx")
            d_slot_s = preload(d_slot, "d_slot")
            p_recip_s = preload(p_recip, "p_recip")
            d_recip_s = preload(d_recip, "d_recip")

            # ---- edge index rebuild + cluster passthrough ----
            cu_t = apool.tile([128, ET], i32, tag="cu")
            cv_t = apool.tile([128, ET], i32, tag="cv")
            nc.sync.dma_start(out=cu_t[:], in_=cu_in[:, :])
            nc.sync.dma_start(out=cv_t[:], in_=cv_in[:, :])
            eq = apool.tile([128, ET], i32, tag="eq")
            nc.vector.tensor_tensor(out=eq[:], in0=cu_t[:], in1=cv_t[:],
                                    op=mybir.AluOpType.is_equal)
            neg1 = apool.tile([128, ET], i32, tag="neg1")
            nc.vector.memset(neg1[:], -1)
            r0 = apool.tile([128, ET], i32, tag="r0")
            nc.vector.select(out=r0[:], mask=eq[:], on_true=neg1[:], on_false=cu_t[:])
            nc.sync.dma_start(out=out_e[0, :, :], in_=r0[:])
            r1 = apool.tile([128, ET], i32, tag="r1")
            nc.vector.select(out=r1[:], mask=eq[:], on_true=neg1[:], on_false=cv_t[:])
            nc.sync.dma_start(out=out_e[1, :, :], in_=r1[:])

            cl_t = apool.tile([128, NT], i32, tag="cl")
            nc.sync.dma_start(out=cl_t[:], in_=cl_in[:, :])
            nc.sync.dma_start(out=out_cl[:, :], in_=cl_t[:])

            # ---- shard column-sum (for giant cluster), DVE accumulate ----
            pxv = pxs[:, :].rearrange("(b t p) f -> b p t f", p=128, t=4)
            nb = pxv.shape[0]
            acc = cpool.tile([128, 512], f32, tag="acc")
            for b in range(nb):
                xt = wpool.tile([128, 512], f32, tag="cs_x")
                nc.sync.dma_start(out=xt[:], in_=pxv[b, :, :, :])
                if b == 0:
                    nc.vector.tensor_copy(out=acc[:], in_=xt[:])
                else:
                    nc.vector.tensor_add(out=acc[:], in0=acc[:], in1=xt[:])
            ps_cs = psa.tile([1, 512], f32, tag="ps_cs")
            nc.tensor.matmul(out=ps_cs[:], lhsT=ones[:], rhs=acc[:],
                             start=True, stop=True)
            stat_tot = apool.tile([1, 128], f32, tag="stat_tot")
            nc.vector.tensor_copy(out=stat_tot[:], in_=ps_cs[0:1, 0:128])
            for blk in range(1, 4):
                nc.vector.tensor_add(out=stat_tot[:], in0=stat_tot[:],
                                     in1=ps_cs[0:1, blk * 128:(blk + 1) * 128])
            nc.sync.dma_start(out=out_st[0:1, :], in_=stat_tot[:])

            # ---- segment-sum groups ----
            def gather(xt_ap, src, idx_col_ap, bound):
                nc.gpsimd.indirect_dma_start(
                    out=xt_ap, out_offset=None, in_=src[:, :],
                    in_offset=bass.IndirectOffsetOnAxis(ap=idx_col_ap, axis=0),
                    bounds_check=bound, oob_is_err=False)

            ps_sm = psa.tile([1, 128], f32, tag="ps_sm")
            n_pm = Gp * Tmp  # total primal member tiles (for small colsum)

            def side(G, Tm, gidx_s, slot_s, recip_s, out_t, src, bound, track_sum):
                mdone = 0
                for g in range(G):
                    ps = psg.tile([128, F], f32, tag="ps_g")
                    for m in range(Tm):
                        j = g * Tm + m
                        xt = wpool.tile([128, F], f32, tag="seg_x")
                        gather(xt[:], src, gidx_s[:, j:j + 1], bound)
                        sm = wpool.tile([128, 128], f32, tag="sel")
                        nc.vector.tensor_tensor(
                            out=sm[:],
                            in0=slot_s[:, j:j + 1].to_broadcast([128, 128]),
                            in1=iota_f[:], op=mybir.AluOpType.is_equal)
                        nc.tensor.matmul(out=ps[:], lhsT=sm[:], rhs=xt[:],
                                         start=(m == 0), stop=(m == Tm - 1))
                        if track_sum:
                            nc.tensor.matmul(out=ps_sm[:], lhsT=ones[:], rhs=xt[:],
                                             start=(mdone == 0),
                                             stop=(mdone == n_pm - 1))
                            mdone += 1
                    o = wpool.tile([128, F], f32, tag="seg_o")
                    nc.vector.tensor_scalar_mul(o[:], ps[:], recip_s[:, g:g + 1])
                    nc.sync.dma_start(out=out_t[g * 128:(g + 1) * 128, :], in_=o[:])

            side(Gp, Tmp, p_gidx_s, p_slot_s, p_recip_s, out_p, pxf, NP_FULL - 1, True)
            side(Gd, Tmd, d_gidx_s, d_slot_s, d_recip_s, out_d, dxf, EP_FULL - 1, False)

            stat_sml = apool.tile([1, 128], f32, tag="stat_sml")
            nc.vector.tensor_copy(out=stat_sml[:], in_=ps_sm[0:1, :])
            nc.sync.dma_start(out=out_st[1:2, :], in_=stat_sml[:])

    nc.compile()
    return nc


# --------------------------------------------------------------------------
# main entry
# --------------------------------------------------------------------------

def kernel(primal_x, dual_x, att, primal_edge_index):
    global _last_results
    primal_x = np.asarray(primal_x, dtype=np.float32)
    dual_x = np.asarray(dual_x, dtype=np.float32)
    att = np.asarray(att, dtype=np.float32)
    pei = np.asarray(primal_edge_index, dtype=np.int32)
    src = pei[0].astype(np.int64)
    dst = pei[1].astype(np.int64)

    # ---------------- host graph bookkeeping ----------------
    mask = _pool_mask(att)
    labels = _connected_components(src, dst, mask)
    uniq, cluster = np.unique(labels, return_inverse=True)
    cluster = cluster.astype(np.int64)
    C = len(uniq)
    counts = np.bincount(cluster, minlength=C).astype(np.float64)
    giant = int(np.argmax(counts))

    cu = cluster[src]
    cv = cluster[dst]
    valid = cu != cv
    va = np.minimum(cu, cv)[valid]
    vb = np.maximum(cu, cv)[valid]
    vidx = np.nonzero(valid)[0]
    upairs, dinv = np.unique(np.stack([va, vb], axis=1), axis=0, return_inverse=True)
    D = len(upairs)
    dcounts = np.bincount(dinv, minlength=D).astype(np.float64)

    p_items = np.array([c for c in range(C) if c != giant], dtype=np.int64)
    members_all = _group_members(cluster, C)
    p_members = [members_all[c] for c in p_items]
    d_items = np.arange(D, dtype=np.int64)
    d_groups = _group_members(dinv, D)
    d_members = [vidx[g] for g in d_groups]  # global edge ids

    # padded full arrays: rows [N, NP_FULL) are zeros -> safe pad gather target
    sp = _Side(p_items, p_members, N)
    sd = _Side(d_items, d_members, E)
    Gp, Tmp, Gd, Tmd = sp.G, sp.Tm, sd.G, sd.Tm

    # ---------------- per-core input maps ----------------
    pxf = np.zeros((NP_FULL, F), dtype=np.float32)
    pxf[:N] = primal_x
    dxf = np.zeros((EP_FULL, F), dtype=np.float32)
    dxf[:E] = dual_x
    iota128 = np.broadcast_to(np.arange(128, dtype=np.float32), (128, 128)).copy()
    in_maps = []
    for k in range(W):
        pxs = np.zeros((NSP, F), dtype=np.float32)
        pxs[:NS] = primal_x[k * NS:(k + 1) * NS]
        cus = np.zeros(ESP, dtype=np.int32)
        cvs = np.zeros(ESP, dtype=np.int32)
        cus[:ES] = cu[k * ES:(k + 1) * ES]
        cvs[:ES] = cv[k * ES:(k + 1) * ES]
        cls = np.zeros(NSP, dtype=np.int32)
        cls[:NS] = cluster[k * NS:(k + 1) * NS]
        in_maps.append({
            "pxf": pxf, "dxf": dxf, "pxs": pxs,
            "p_gidx": _wrap128(sp.gidx[k]),
            "p_slot": _wrap128(sp.slot[k]),
            "d_gidx": _wrap128(sd.gidx[k]),
            "d_slot": _wrap128(sd.slot[k]),
            "p_recip": _wrap128(sp.recips(k, counts[p_items]).astype(np.float32)),
            "d_recip": _wrap128(sd.recips(k, dcounts).astype(np.float32)),
            "cu": _wrap128(cus), "cv": _wrap128(cvs), "cl": _wrap128(cls),
            "iota": iota128,
        })

    # ---------------- build + run ----------------
    nc = _build_program(Gp, Tmp, Gd, Tmd, batched=GATHER_BATCH > 1)
    res = run_bass_kernel_spmd(nc, in_maps, list(range(W)))
    _last_results = res

    # ---------------- host assembly ----------------
    new_primal_x = np.zeros((N, F), dtype=np.float32)
    new_dual_x = np.zeros((E, F), dtype=np.float32)
    tot = np.zeros(F, dtype=np.float64)
    sml = np.zeros(F, dtype=np.float64)
    for k in range(W):
        o = sp.owned[k]
        if len(o):
            new_primal_x[p_items[o]] = res.results[k]["out_p"][: len(o)]
        od = sd.owned[k]
        if len(od):
            new_dual_x[d_items[od]] = res.results[k]["out_d"][: len(od)]
        st = res.results[k]["out_st"]
        tot += st[0]
        sml += st[1]
    new_primal_x[giant] = ((tot - sml) / max(counts[giant], 1.0)).astype(np.float32)

    npei = np.empty((2, E), dtype=np.int32)
    for k in range(W):
        oe = res.results[k]["out_e"]  # [2, 128, ET] wrapped
        npei[0, k * ES:(k + 1) * ES] = oe[0].T.reshape(-1)[:ES]
        npei[1, k * ES:(k + 1) * ES] = oe[1].T.reshape(-1)[:ES]

    cl_out = np.empty(N, dtype=np.int32)
    for k in range(W):
        cl_out[k * NS:(k + 1) * NS] = res.results[k]["out_cl"].T.reshape(-1)[:NS]

    return new_primal_x, new_dual_x, npei, cl_out


# revision 19
# speedup vs baseline: 2.5675x; 1.0324x over previous
"""DualPrimalEdgePooling on 8 TRN2 NeuronCores.

Strategy (graph/data parallel, collective-free):
  - Host computes the O(E) int32 bookkeeping: top-k pool mask, connected
    components (union-find by min label), cluster compaction, dual-pair
    compaction, and the gather/packing tables that drive the device.
  - The 8-core SPMD Bass kernel does all O(N*F)/O(E*F) feature work with
    fully independent cores (collectives on this part measure ~70-110us of
    fixed barrier cost, so ownership is arranged to need none):
      * primal_x/dual_x are visible to every core; segment ownership is
        round-robin.  Each core indirect-DMA gathers the member rows of its
        owned segments, reduces them with one-hot (iota==slot) selection
        matmuls accumulated in PSUM, applies the mean via a per-partition
        reciprocal scale, and writes compact outputs.
      * the giant component's sum is (total - sum of small members): each
        core column-sum reduces one contiguous 1/8 shard of primal_x (DVE
        accumulate + one ones-matmul) and the colsum of its gathered small
        members, emitting a [2,128] partial; the final (tot-small)/count for
        that single output row is folded on the host during unsharding.
      * new_primal_edge_index is rebuilt on-device (is_equal + select),
        cluster passes through the device.
  - Host scatters the compact device outputs into the full-size (mostly
    zero) result tensors.
"""

import os
import sys

for _p in ("/opt/trn_rl_repo",):
    if _p not in sys.path and os.path.isdir(_p):
        sys.path.insert(0, _p)

import numpy as np

import concourse.bass as bass
import concourse.bacc as bacc
import concourse.mybir as mybir
from concourse.bass_utils import run_bass_kernel_spmd
from concourse.tile import TileContext

N = 200_000
E = 600_000
F = 128
NUM_KEEP = 300_000
W = 8                     # cores
NS = N // W               # primal rows per shard  (25000)
ES = E // W               # dual rows / edges per shard (75000)
NSP = ((NS + 127) // 128) * 128   # padded shard rows (25088)
ESP = ((ES + 127) // 128) * 128   # padded shard rows (75008)
NP_FULL = NSP * W                 # padded full primal rows (200704)
EP_FULL = ESP * W                 # padded full dual rows (600064)
GATHER_BATCH = 1                  # offset columns per indirect DMA (multi-column
                                  # offsets gather garbage — verified on HW)

_last_results = None      # BassKernelResults of the most recent run (for test harness)


# --------------------------------------------------------------------------
# host-side graph bookkeeping
# --------------------------------------------------------------------------

def _pool_mask(att):
    order = np.argsort(-att, kind="stable")
    m = np.zeros(E, dtype=bool)
    m[order[: E - NUM_KEEP]] = True
    return m


def _connected_components(src, dst, mask):
    """labels[i] = min node id in i's component over masked edges."""
    parent = np.arange(N, dtype=np.int64)

    def find(a):
        while parent[a] != a:
            parent[a] = parent[parent[a]]
            a = parent[a]
        return a

    for a, b in zip(src[mask], dst[mask]):
        ra, rb = find(a), find(b)
        if ra != rb:
            if ra < rb:
                parent[rb] = ra
            else:
                parent[ra] = rb
    lab = np.empty(N, dtype=np.int64)
    for i in range(N):
        lab[i] = find(i)
    return lab


def _group_members(keys, num_keys):
    """For int array keys (>=0), return list-of-arrays members per key value."""
    order = np.argsort(keys, kind="stable")
    sk = keys[order]
    starts = np.searchsorted(sk, np.arange(num_keys), side="left")
    ends = np.searchsorted(sk, np.arange(num_keys), side="right")
    return [order[s:e] for s, e in zip(starts, ends)]


class _Side:
    """Packing tables for one segment-sum side (primal clusters / dual segs).

    Items are assigned round-robin to cores. Within a core, multi-member
    items come first: groups [0, G_mm) reduce via one-hot selection matmuls
    with exact per-group tile counts Tm_list[g]; groups [G_mm, G) hold only
    singletons, whose output row IS the gathered member row (no matmul).
    All group/tile counts are maxima across cores so the SPMD program is
    uniform; padding gathers a known zero row."""

    def __init__(self, item_ids, members_per_item, pad_row):
        self.item_ids = item_ids
        M = len(item_ids)
        owner = np.arange(M) % W
        sizes = np.array([len(members_per_item[i]) for i in range(M)], dtype=np.int64)
        self.owned = []
        n_multi = []
        for k in range(W):
            o = np.nonzero(owner == k)[0]
            multi = o[sizes[o] > 1]
            single = o[sizes[o] <= 1]
            self.owned.append(np.concatenate([multi, single]))
            n_multi.append(len(multi))
        self.G = max(1, max((len(o) + 127) // 128 for o in self.owned))
        self.G_mm = max((nm + 127) // 128 for nm in n_multi)
        if self.G_mm == 0 and max(n_multi) == 0:
            self.G_mm = 0
        # exact per-matmul-group tile counts (max across cores)
        rows_kg = np.zeros((W, max(self.G_mm, 1)), dtype=np.int64)
        for k in range(W):
            reg = self.owned[k][: self.G_mm * 128]
            for gi, ii in enumerate(reg):
                rows_kg[k, gi // 128] += sizes[ii]
        self.Tm_list = [max(1, int(np.max((rows_kg[:, g] + 127) // 128)))
                        for g in range(self.G_mm)]
        self.n_mm_tiles = sum(self.Tm_list)
        ncols = self.n_mm_tiles + (self.G - self.G_mm)
        self.ncols = ncols
        self.gidx = []
        self.slot = []
        tile_base = np.cumsum([0] + self.Tm_list)  # start tile of mm group g
        for k in range(W):
            gi = np.full(ncols * 128, pad_row, dtype=np.int64)
            sl = np.zeros(self.n_mm_tiles * 128, dtype=np.float32)
            fill = np.zeros(max(self.G_mm, 1), dtype=np.int64)
            for gidx_i, ii in enumerate(self.owned[k]):
                g, slot = divmod(gidx_i, 128)
                if g < self.G_mm:
                    for gid in members_per_item[ii]:
                        pos = tile_base[g] * 128 + fill[g]
                        gi[pos] = gid
                        sl[pos] = slot
                        fill[g] += 1
                else:
                    pos = (self.n_mm_tiles + (g - self.G_mm)) * 128 + slot
                    gi[pos] = members_per_item[ii][0]
            self.gidx.append(gi.astype(np.int32))
            self.slot.append(sl)

    def recips(self, k, counts_per_item):
        r = np.ones(self.G * 128, dtype=np.float32)
        o = self.owned[k]
        r[: len(o)] = 1.0 / np.maximum(counts_per_item[o], 1.0)
        return r


def _wrap128(a):
    """[T*128] -> [128, T] transposed tile layout (column t = tile t)."""
    assert a.size % 128 == 0
    return np.ascontiguousarray(a.reshape(-1, 128).T)


# --------------------------------------------------------------------------
# device program
# --------------------------------------------------------------------------

def _build_program(sp, sd):
    f32, i32 = mybir.dt.float32, mybir.dt.int32
    nc = bacc.Bacc(None, target_bir_lowering=False)

    Gp = sp.G
    Gd = sd.G
    pxf = nc.declare_dram_parameter("pxf", [NP_FULL, F], f32, isOutput=False)
    dxf = nc.declare_dram_parameter("dxf", [EP_FULL, F], f32, isOutput=False)
    pxs = nc.declare_dram_parameter("pxs", [NSP, F], f32, isOutput=False)
    p_gidx = nc.declare_dram_parameter("p_gidx", [128, sp.ncols], i32, isOutput=False)
    p_slot = nc.declare_dram_parameter("p_slot", [128, max(sp.n_mm_tiles, 1)], f32,
                                       isOutput=False)
    d_gidx = nc.declare_dram_parameter("d_gidx", [128, sd.ncols], i32, isOutput=False)
    d_slot = nc.declare_dram_parameter("d_slot", [128, max(sd.n_mm_tiles, 1)], f32,
                                       isOutput=False)
    p_recip = nc.declare_dram_parameter("p_recip", [128, Gp], f32, isOutput=False)
    d_recip = nc.declare_dram_parameter("d_recip", [128, Gd], f32, isOutput=False)
    cu_in = nc.declare_dram_parameter("cu", [128, ESP // 128], i32, isOutput=False)
    cv_in = nc.declare_dram_parameter("cv", [128, ESP // 128], i32, isOutput=False)
    cl_in = nc.declare_dram_parameter("cl", [128, NSP // 128], i32, isOutput=False)
    iota_in = nc.declare_dram_parameter("iota", [128, 128], f32, isOutput=False)

    out_p = nc.declare_dram_parameter("out_p", [Gp * 128, F], f32, isOutput=True)
    out_d = nc.declare_dram_parameter("out_d", [Gd * 128, F], f32, isOutput=True)
    out_e = nc.declare_dram_parameter("out_e", [2, 128, ESP // 128], i32, isOutput=True)
    out_cl = nc.declare_dram_parameter("out_cl", [128, NSP // 128], i32, isOutput=True)
    out_st = nc.declare_dram_parameter("out_st", [2, F], f32, isOutput=True)

    ET = ESP // 128
    NT = NSP // 128

    with TileContext(nc) as tc:
        with (
            tc.tile_pool(name="const", bufs=1) as cpool,
            tc.tile_pool(name="work", bufs=6) as wpool,
            tc.tile_pool(name="aux", bufs=1) as apool,
            tc.tile_pool(name="psa", bufs=1, space="PSUM") as psa,
            tc.tile_pool(name="psg", bufs=3, space="PSUM") as psg,
        ):
            # ---- constants / preloaded tables ----
            ones = cpool.tile([128, 1], f32, tag="ones")
            nc.vector.memset(ones[:], 1.0)
            iota_f = cpool.tile([128, 128], f32, tag="iota")
            nc.sync.dma_start(out=iota_f[:], in_=iota_in[:, :])

            def preload(t, tag):
                s = cpool.tile(list(t.shape), t.dtype, tag=tag)
                nc.sync.dma_start(out=s[:], in_=t[:, :])
                return s

            p_gidx_s = preload(p_gidx, "p_gidx")
            p_slot_s = preload(p_slot, "p_slot")
            d_gidx_s = preload(d_gidx, "d_gidx")
            d_slot_s = preload(d_slot, "d_slot")
            p_recip_s = preload(p_recip, "p_recip")
            d_recip_s = preload(d_recip, "d_recip")

            # ---- edge index rebuild + cluster passthrough ----
            cu_t = apool.tile([128, ET], i32, tag="cu")
            cv_t = apool.tile([128, ET], i32, tag="cv")
            nc.sync.dma_start(out=cu_t[:], in_=cu_in[:, :])
            nc.sync.dma_start(out=cv_t[:], in_=cv_in[:, :])
            eq = apool.tile([128, ET], i32, tag="eq")
            nc.vector.tensor_tensor(out=eq[:], in0=cu_t[:], in1=cv_t[:],
                                    op=mybir.AluOpType.is_equal)
            neg1 = apool.tile([128, ET], i32, tag="neg1")
            nc.vector.memset(neg1[:], -1)
            r0 = apool.tile([128, ET], i32, tag="r0")
            nc.vector.select(out=r0[:], mask=eq[:], on_true=neg1[:], on_false=cu_t[:])
            nc.sync.dma_start(out=out_e[0, :, :], in_=r0[:])
            r1 = apool.tile([128, ET], i32, tag="r1")
            nc.vector.select(out=r1[:], mask=eq[:], on_true=neg1[:], on_false=cv_t[:])
            nc.sync.dma_start(out=out_e[1, :, :], in_=r1[:])

            cl_t = apool.tile([128, NT], i32, tag="cl")
            nc.sync.dma_start(out=cl_t[:], in_=cl_in[:, :])
            nc.sync.dma_start(out=out_cl[:, :], in_=cl_t[:])

            # ---- shard column-sum (for giant cluster), DVE accumulate ----
            pxv = pxs[:, :].rearrange("(b t p) f -> b p t f", p=128, t=4)
            nb = pxv.shape[0]
            acc = cpool.tile([128, 512], f32, tag="acc")
            for b in range(nb):
                xt = wpool.tile([128, 512], f32, tag="cs_x")
                nc.sync.dma_start(out=xt[:], in_=pxv[b, :, :, :])
                if b == 0:
                    nc.vector.tensor_copy(out=acc[:], in_=xt[:])
                else:
                    nc.vector.tensor_add(out=acc[:], in0=acc[:], in1=xt[:])
            ps_cs = psa.tile([1, 512], f32, tag="ps_cs")
            nc.tensor.matmul(out=ps_cs[:], lhsT=ones[:], rhs=acc[:],
                             start=True, stop=True)
            stat_tot = apool.tile([1, 128], f32, tag="stat_tot")
            nc.vector.tensor_copy(out=stat_tot[:], in_=ps_cs[0:1, 0:128])
            for blk in range(1, 4):
                nc.vector.tensor_add(out=stat_tot[:], in0=stat_tot[:],
                                     in1=ps_cs[0:1, blk * 128:(blk + 1) * 128])
            nc.sync.dma_start(out=out_st[0:1, :], in_=stat_tot[:])

            # ---- segment-sum groups ----
            def gather(xt_ap, src, idx_col_ap, bound):
                nc.gpsimd.indirect_dma_start(
                    out=xt_ap, out_offset=None, in_=src[:, :],
                    in_offset=bass.IndirectOffsetOnAxis(ap=idx_col_ap, axis=0),
                    bounds_check=bound, oob_is_err=False)

            # small-member colsum accumulator (primal side only), DVE chain
            acc_sml = cpool.tile([128, 128], f32, tag="acc_sml")
            sml_state = {"n": 0}

            def track(xt):
                if sml_state["n"] == 0:
                    nc.vector.tensor_copy(out=acc_sml[:], in_=xt[:])
                else:
                    nc.vector.tensor_add(out=acc_sml[:], in0=acc_sml[:], in1=xt[:])
                sml_state["n"] += 1

            def side(S, gidx_s, slot_s, recip_s, out_t, src, bound, track_sum):
                tile_base = [0]
                for t in S.Tm_list:
                    tile_base.append(tile_base[-1] + t)
                # matmul groups
                for g in range(S.G_mm):
                    ps = psg.tile([128, F], f32, tag="ps_g")
                    Tm = S.Tm_list[g]
                    for m in range(Tm):
                        j = tile_base[g] + m
                        xt = wpool.tile([128, F], f32, tag="seg_x")
                        gather(xt[:], src, gidx_s[:, j:j + 1], bound)
                        sm = wpool.tile([128, 128], f32, tag="sel")
                        nc.vector.tensor_tensor(
                            out=sm[:],
                            in0=slot_s[:, j:j + 1].to_broadcast([128, 128]),
                            in1=iota_f[:], op=mybir.AluOpType.is_equal)
                        nc.tensor.matmul(out=ps[:], lhsT=sm[:], rhs=xt[:],
                                         start=(m == 0), stop=(m == Tm - 1))
                        if track_sum:
                            track(xt)
                    o = wpool.tile([128, F], f32, tag="seg_o")
                    nc.vector.tensor_scalar_mul(o[:], ps[:], recip_s[:, g:g + 1])
                    nc.sync.dma_start(out=out_t[g * 128:(g + 1) * 128, :], in_=o[:])
                # singleton groups: output row = gathered member row * recip
                for g in range(S.G_mm, S.G):
                    j = S.n_mm_tiles + (g - S.G_mm)
                    xt = wpool.tile([128, F], f32, tag="seg_x")
                    gather(xt[:], src, gidx_s[:, j:j + 1], bound)
                    if track_sum:
                        track(xt)
                    o = wpool.tile([128, F], f32, tag="seg_o")
                    nc.vector.tensor_scalar_mul(o[:], xt[:], recip_s[:, g:g + 1])
                    nc.sync.dma_start(out=out_t[g * 128:(g + 1) * 128, :], in_=o[:])

            side(sp, p_gidx_s, p_slot_s, p_recip_s, out_p, pxf, NP_FULL - 1, True)
            side(sd, d_gidx_s, d_slot_s, d_recip_s, out_d, dxf, EP_FULL - 1, False)

            ps_sm = psa.tile([1, 128], f32, tag="ps_sm")
            nc.tensor.matmul(out=ps_sm[:], lhsT=ones[:], rhs=acc_sml[:],
                             start=True, stop=True)
            stat_sml = apool.tile([1, 128], f32, tag="stat_sml")
            nc.vector.tensor_copy(out=stat_sml[:], in_=ps_sm[0:1, :])
            nc.sync.dma_start(out=out_st[1:2, :], in_=stat_sml[:])

    nc.compile()
    return nc


# --------------------------------------------------------------------------
# main entry
# --------------------------------------------------------------------------

def kernel(primal_x, dual_x, att, primal_edge_index):
    global _last_results
    primal_x = np.asarray(primal_x, dtype=np.float32)
    dual_x = np.asarray(dual_x, dtype=np.float32)
    att = np.asarray(att, dtype=np.float32)
    pei = np.asarray(primal_edge_index, dtype=np.int32)
    src = pei[0].astype(np.int64)
    dst = pei[1].astype(np.int64)

    # ---------------- host graph bookkeeping ----------------
    mask = _pool_mask(att)
    labels = _connected_components(src, dst, mask)
    uniq, cluster = np.unique(labels, return_inverse=True)
    cluster = cluster.astype(np.int64)
    C = len(uniq)
    counts = np.bincount(cluster, minlength=C).astype(np.float64)
    giant = int(np.argmax(counts))

    cu = cluster[src]
    cv = cluster[dst]
    valid = cu != cv
    va = np.minimum(cu, cv)[valid]
    vb = np.maximum(cu, cv)[valid]
    vidx = np.nonzero(valid)[0]
    upairs, dinv = np.unique(np.stack([va, vb], axis=1), axis=0, return_inverse=True)
    D = len(upairs)
    dcounts = np.bincount(dinv, minlength=D).astype(np.float64)

    p_items = np.array([c for c in range(C) if c != giant], dtype=np.int64)
    members_all = _group_members(cluster, C)
    p_members = [members_all[c] for c in p_items]
    d_items = np.arange(D, dtype=np.int64)
    d_groups = _group_members(dinv, D)
    d_members = [vidx[g] for g in d_groups]  # global edge ids

    # padded full arrays: rows [N, NP_FULL) are zeros -> safe pad gather target
    sp = _Side(p_items, p_members, N)
    sd = _Side(d_items, d_members, E)

    # ---------------- per-core input maps ----------------
    pxf = np.zeros((NP_FULL, F), dtype=np.float32)
    pxf[:N] = primal_x
    dxf = np.zeros((EP_FULL, F), dtype=np.float32)
    dxf[:E] = dual_x
    iota128 = np.broadcast_to(np.arange(128, dtype=np.float32), (128, 128)).copy()
    in_maps = []
    for k in range(W):
        pxs = np.zeros((NSP, F), dtype=np.float32)
        pxs[:NS] = primal_x[k * NS:(k + 1) * NS]
        cus = np.zeros(ESP, dtype=np.int32)
        cvs = np.zeros(ESP, dtype=np.int32)
        cus[:ES] = cu[k * ES:(k + 1) * ES]
        cvs[:ES] = cv[k * ES:(k + 1) * ES]
        cls = np.zeros(NSP, dtype=np.int32)
        cls[:NS] = cluster[k * NS:(k + 1) * NS]
        def slot_arr(s):
            a = s.slot[k]
            return _wrap128(a if a.size else np.zeros(128, np.float32))

        in_maps.append({
            "pxf": pxf, "dxf": dxf, "pxs": pxs,
            "p_gidx": _wrap128(sp.gidx[k]),
            "p_slot": slot_arr(sp),
            "d_gidx": _wrap128(sd.gidx[k]),
            "d_slot": slot_arr(sd),
            "p_recip": _wrap128(sp.recips(k, counts[p_items]).astype(np.float32)),
            "d_recip": _wrap128(sd.recips(k, dcounts).astype(np.float32)),
            "cu": _wrap128(cus), "cv": _wrap128(cvs), "cl": _wrap128(cls),
            "iota": iota128,
        })

    # ---------------- build + run ----------------
    nc = _build_program(sp, sd)
    res = run_bass_kernel_spmd(nc, in_maps, list(range(W)))
    _last_results = res

    # ---------------- host assembly ----------------
    new_primal_x = np.zeros((N, F), dtype=np.float32)
    new_dual_x = np.zeros((E, F), dtype=np.float32)
    tot = np.zeros(F, dtype=np.float64)
    sml = np.zeros(F, dtype=np.float64)
    for k in range(W):
        o = sp.owned[k]
        if len(o):
            new_primal_x[p_items[o]] = res.results[k]["out_p"][: len(o)]
        od = sd.owned[k]
        if len(od):
            new_dual_x[d_items[od]] = res.results[k]["out_d"][: len(od)]
        st = res.results[k]["out_st"]
        tot += st[0]
        sml += st[1]
    new_primal_x[giant] = ((tot - sml) / max(counts[giant], 1.0)).astype(np.float32)

    npei = np.empty((2, E), dtype=np.int32)
    for k in range(W):
        oe = res.results[k]["out_e"]  # [2, 128, ET] wrapped
        npei[0, k * ES:(k + 1) * ES] = oe[0].T.reshape(-1)[:ES]
        npei[1, k * ES:(k + 1) * ES] = oe[1].T.reshape(-1)[:ES]

    cl_out = np.empty(N, dtype=np.int32)
    for k in range(W):
        cl_out[k * NS:(k + 1) * NS] = res.results[k]["out_cl"].T.reshape(-1)[:NS]

    return new_primal_x, new_dual_x, npei, cl_out


# revision 23
# speedup vs baseline: 3.7420x; 1.4574x over previous
"""DualPrimalEdgePooling on 8 TRN2 NeuronCores.

Strategy (graph/data parallel, collective-free):
  - Host computes the O(E) int32 bookkeeping: top-k pool mask, connected
    components (union-find by min label), cluster compaction, dual-pair
    compaction, and the gather/packing tables that drive the device.
  - The 8-core SPMD Bass kernel does all O(N*F)/O(E*F) feature work with
    fully independent cores (collectives on this part measure ~70-110us of
    fixed barrier cost, so ownership is arranged to need none):
      * primal_x/dual_x are visible to every core; segment ownership is
        round-robin.  Each core indirect-DMA gathers the member rows of its
        owned segments, reduces them with one-hot (iota==slot) selection
        matmuls accumulated in PSUM, applies the mean via a per-partition
        reciprocal scale, and writes compact outputs.
      * the giant component's sum is (total - sum of small members): each
        core column-sum reduces one contiguous 1/8 shard of primal_x (DVE
        accumulate + one ones-matmul) and the colsum of its gathered small
        members, emitting a [2,128] partial; the final (tot-small)/count for
        that single output row is folded on the host during unsharding.
      * new_primal_edge_index is rebuilt on-device (is_equal + select),
        cluster passes through the device.
  - Host scatters the compact device outputs into the full-size (mostly
    zero) result tensors.
"""

import os
import sys

for _p in ("/opt/trn_rl_repo",):
    if _p not in sys.path and os.path.isdir(_p):
        sys.path.insert(0, _p)

import numpy as np

import concourse.bass as bass
import concourse.bacc as bacc
import concourse.mybir as mybir
from concourse.bass_utils import run_bass_kernel_spmd
from concourse.tile import TileContext

N = 200_000
E = 600_000
F = 128
NUM_KEEP = 300_000
W = 8                     # cores
NS = N // W               # primal rows per shard  (25000)
ES = E // W               # dual rows / edges per shard (75000)
NSP = ((NS + 127) // 128) * 128   # padded shard rows (25088)
ESP = ((ES + 127) // 128) * 128   # padded shard rows (75008)
NP_FULL = NSP * W                 # padded full primal rows (200704)
EP_FULL = ESP * W                 # padded full dual rows (600064)
GATHER_BATCH = 1                  # offset columns per indirect DMA (multi-column
                                  # offsets gather garbage — verified on HW)

_last_results = None      # BassKernelResults of the most recent run (for test harness)


# --------------------------------------------------------------------------
# host-side graph bookkeeping
# --------------------------------------------------------------------------

def _pool_mask(att):
    order = np.argsort(-att, kind="stable")
    m = np.zeros(E, dtype=bool)
    m[order[: E - NUM_KEEP]] = True
    return m


def _connected_components(src, dst, mask):
    """labels[i] = min node id in i's component over masked edges."""
    parent = np.arange(N, dtype=np.int64)

    def find(a):
        while parent[a] != a:
            parent[a] = parent[parent[a]]
            a = parent[a]
        return a

    for a, b in zip(src[mask], dst[mask]):
        ra, rb = find(a), find(b)
        if ra != rb:
            if ra < rb:
                parent[rb] = ra
            else:
                parent[ra] = rb
    lab = np.empty(N, dtype=np.int64)
    for i in range(N):
        lab[i] = find(i)
    return lab


def _group_members(keys, num_keys):
    """For int array keys (>=0), return list-of-arrays members per key value."""
    order = np.argsort(keys, kind="stable")
    sk = keys[order]
    starts = np.searchsorted(sk, np.arange(num_keys), side="left")
    ends = np.searchsorted(sk, np.arange(num_keys), side="right")
    return [order[s:e] for s, e in zip(starts, ends)]


class _Side:
    """Packing tables for one segment-sum side (primal clusters / dual segs).

    Items are assigned round-robin to cores. Within a core, multi-member
    items come first: groups [0, G_mm) reduce via one-hot selection matmuls
    with exact per-group tile counts Tm_list[g]; groups [G_mm, G) hold only
    singletons, whose output row IS the gathered member row (no matmul).
    All group/tile counts are maxima across cores so the SPMD program is
    uniform; padding gathers a known zero row."""

    def __init__(self, item_ids, members_per_item, pad_row):
        self.item_ids = item_ids
        M = len(item_ids)
        owner = np.arange(M) % W
        sizes = np.array([len(members_per_item[i]) for i in range(M)], dtype=np.int64)
        self.owned = []
        n_multi = []
        for k in range(W):
            o = np.nonzero(owner == k)[0]
            multi = o[sizes[o] > 1]
            # size-ascending keeps per-group member counts similar across
            # cores, so the cross-core max tile count per group stays tight
            multi = multi[np.argsort(sizes[multi], kind="stable")]
            single = o[sizes[o] <= 1]
            self.owned.append(np.concatenate([multi, single]))
            n_multi.append(len(multi))
        self.G = max(1, max((len(o) + 127) // 128 for o in self.owned))
        self.G_mm = max((nm + 127) // 128 for nm in n_multi)
        if self.G_mm == 0 and max(n_multi) == 0:
            self.G_mm = 0
        # exact per-matmul-group tile counts (max across cores)
        rows_kg = np.zeros((W, max(self.G_mm, 1)), dtype=np.int64)
        for k in range(W):
            reg = self.owned[k][: self.G_mm * 128]
            for gi, ii in enumerate(reg):
                rows_kg[k, gi // 128] += sizes[ii]
        self.Tm_list = [max(1, int(np.max((rows_kg[:, g] + 127) // 128)))
                        for g in range(self.G_mm)]
        self.n_mm_tiles = sum(self.Tm_list)
        ncols = self.n_mm_tiles + (self.G - self.G_mm)
        self.ncols = ncols
        self.gidx = []
        self.slot = []
        tile_base = np.cumsum([0] + self.Tm_list)  # start tile of mm group g
        for k in range(W):
            gi = np.full(ncols * 128, pad_row, dtype=np.int64)
            sl = np.zeros(self.n_mm_tiles * 128, dtype=np.float32)
            fill = np.zeros(max(self.G_mm, 1), dtype=np.int64)
            for gidx_i, ii in enumerate(self.owned[k]):
                g, slot = divmod(gidx_i, 128)
                if g < self.G_mm:
                    for gid in members_per_item[ii]:
                        pos = tile_base[g] * 128 + fill[g]
                        gi[pos] = gid
                        sl[pos] = slot
                        fill[g] += 1
                else:
                    pos = (self.n_mm_tiles + (g - self.G_mm)) * 128 + slot
                    gi[pos] = members_per_item[ii][0]
            self.gidx.append(gi.astype(np.int32))
            self.slot.append(sl)

    def recips(self, k, counts_per_item):
        r = np.ones(self.G * 128, dtype=np.float32)
        o = self.owned[k]
        r[: len(o)] = 1.0 / np.maximum(counts_per_item[o], 1.0)
        return r


def _wrap128(a):
    """[T*128] -> [128, T] transposed tile layout (column t = tile t)."""
    assert a.size % 128 == 0
    return np.ascontiguousarray(a.reshape(-1, 128).T)


# --------------------------------------------------------------------------
# device program
# --------------------------------------------------------------------------

def _build_program(sp, sd):
    f32, i32 = mybir.dt.float32, mybir.dt.int32
    nc = bacc.Bacc(None, target_bir_lowering=False)

    Gp = sp.G
    Gd = sd.G
    pxf = nc.declare_dram_parameter("pxf", [NP_FULL, F], f32, isOutput=False)
    dxf = nc.declare_dram_parameter("dxf", [EP_FULL, F], f32, isOutput=False)
    pxs = nc.declare_dram_parameter("pxs", [NSP, F], f32, isOutput=False)
    p_gidx = nc.declare_dram_parameter("p_gidx", [128, sp.ncols], i32, isOutput=False)
    p_slot = nc.declare_dram_parameter("p_slot", [128, max(sp.n_mm_tiles, 1)], f32,
                                       isOutput=False)
    d_gidx = nc.declare_dram_parameter("d_gidx", [128, sd.ncols], i32, isOutput=False)
    d_slot = nc.declare_dram_parameter("d_slot", [128, max(sd.n_mm_tiles, 1)], f32,
                                       isOutput=False)
    p_recip = nc.declare_dram_parameter("p_recip", [128, Gp], f32, isOutput=False)
    d_recip = nc.declare_dram_parameter("d_recip", [128, Gd], f32, isOutput=False)
    cu_in = nc.declare_dram_parameter("cu", [128, ESP // 128], i32, isOutput=False)
    cv_in = nc.declare_dram_parameter("cv", [128, ESP // 128], i32, isOutput=False)
    cl_in = nc.declare_dram_parameter("cl", [128, NSP // 128], i32, isOutput=False)
    iota_in = nc.declare_dram_parameter("iota", [128, 128], f32, isOutput=False)

    out_p = nc.declare_dram_parameter("out_p", [Gp * 128, F], f32, isOutput=True)
    out_d = nc.declare_dram_parameter("out_d", [Gd * 128, F], f32, isOutput=True)
    out_e = nc.declare_dram_parameter("out_e", [2, 128, ESP // 128], i32, isOutput=True)
    out_cl = nc.declare_dram_parameter("out_cl", [128, NSP // 128], i32, isOutput=True)
    out_st = nc.declare_dram_parameter("out_st", [2, F], f32, isOutput=True)

    ET = ESP // 128
    NT = NSP // 128

    with TileContext(nc) as tc:
        with (
            tc.tile_pool(name="const", bufs=1) as cpool,
            tc.tile_pool(name="work", bufs=12) as wpool,
            tc.tile_pool(name="aux", bufs=1) as apool,
            tc.tile_pool(name="psa", bufs=1, space="PSUM") as psa,
            tc.tile_pool(name="psg", bufs=4, space="PSUM") as psg,
        ):
            # ---- constants / preloaded tables ----
            ones = cpool.tile([128, 1], f32, tag="ones")
            nc.vector.memset(ones[:], 1.0)
            iota_f = cpool.tile([128, 128], f32, tag="iota")
            nc.sync.dma_start(out=iota_f[:], in_=iota_in[:, :])

            def preload(t, tag):
                s = cpool.tile(list(t.shape), t.dtype, tag=tag)
                nc.sync.dma_start(out=s[:], in_=t[:, :])
                return s

            p_gidx_s = preload(p_gidx, "p_gidx")
            p_slot_s = preload(p_slot, "p_slot")
            d_gidx_s = preload(d_gidx, "d_gidx")
            d_slot_s = preload(d_slot, "d_slot")
            p_recip_s = preload(p_recip, "p_recip")
            d_recip_s = preload(d_recip, "d_recip")

            # ---- edge index rebuild + cluster passthrough ----
            cu_t = apool.tile([128, ET], i32, tag="cu")
            cv_t = apool.tile([128, ET], i32, tag="cv")
            nc.sync.dma_start(out=cu_t[:], in_=cu_in[:, :])
            nc.sync.dma_start(out=cv_t[:], in_=cv_in[:, :])
            eq = apool.tile([128, ET], i32, tag="eq")
            nc.vector.tensor_tensor(out=eq[:], in0=cu_t[:], in1=cv_t[:],
                                    op=mybir.AluOpType.is_equal)
            neg1 = apool.tile([128, ET], i32, tag="neg1")
            nc.vector.memset(neg1[:], -1)
            r0 = apool.tile([128, ET], i32, tag="r0")
            nc.vector.select(out=r0[:], mask=eq[:], on_true=neg1[:], on_false=cu_t[:])
            nc.sync.dma_start(out=out_e[0, :, :], in_=r0[:])
            r1 = apool.tile([128, ET], i32, tag="r1")
            nc.vector.select(out=r1[:], mask=eq[:], on_true=neg1[:], on_false=cv_t[:])
            nc.sync.dma_start(out=out_e[1, :, :], in_=r1[:])

            cl_t = apool.tile([128, NT], i32, tag="cl")
            nc.sync.dma_start(out=cl_t[:], in_=cl_in[:, :])
            nc.sync.dma_start(out=out_cl[:, :], in_=cl_t[:])

            # ---- shard column-sum (for giant cluster), DVE accumulate ----
            pxv = pxs[:, :].rearrange("(b t p) f -> b p t f", p=128, t=4)
            nb = pxv.shape[0]
            acc = cpool.tile([128, 512], f32, tag="acc")
            for b in range(nb):
                xt = wpool.tile([128, 512], f32, tag="cs_x")
                nc.sync.dma_start(out=xt[:], in_=pxv[b, :, :, :])
                if b == 0:
                    nc.vector.tensor_copy(out=acc[:], in_=xt[:])
                else:
                    nc.vector.tensor_add(out=acc[:], in0=acc[:], in1=xt[:])
            ps_cs = psa.tile([1, 512], f32, tag="ps_cs")
            nc.tensor.matmul(out=ps_cs[:], lhsT=ones[:], rhs=acc[:],
                             start=True, stop=True)
            stat_tot = apool.tile([1, 128], f32, tag="stat_tot")
            nc.vector.tensor_copy(out=stat_tot[:], in_=ps_cs[0:1, 0:128])
            for blk in range(1, 4):
                nc.vector.tensor_add(out=stat_tot[:], in0=stat_tot[:],
                                     in1=ps_cs[0:1, blk * 128:(blk + 1) * 128])
            nc.sync.dma_start(out=out_st[0:1, :], in_=stat_tot[:])

            # ---- segment-sum groups ----
            def gather(xt_ap, src, idx_col_ap, bound):
                nc.gpsimd.indirect_dma_start(
                    out=xt_ap, out_offset=None, in_=src[:, :],
                    in_offset=bass.IndirectOffsetOnAxis(ap=idx_col_ap, axis=0),
                    bounds_check=bound, oob_is_err=False)

            # small-member colsum accumulator (primal side only), DVE chain
            acc_sml = cpool.tile([128, 128], f32, tag="acc_sml")
            sml_state = {"n": 0}

            def track(xt):
                if sml_state["n"] == 0:
                    nc.vector.tensor_copy(out=acc_sml[:], in_=xt[:])
                else:
                    nc.vector.tensor_add(out=acc_sml[:], in0=acc_sml[:], in1=xt[:])
                sml_state["n"] += 1

            def side(S, gidx_s, slot_s, recip_s, out_t, src, bound, track_sum):
                tile_base = [0]
                for t in S.Tm_list:
                    tile_base.append(tile_base[-1] + t)
                # matmul groups
                for g in range(S.G_mm):
                    ps = psg.tile([128, F], f32, tag="ps_g")
                    Tm = S.Tm_list[g]
                    for m in range(Tm):
                        j = tile_base[g] + m
                        xt = wpool.tile([128, F], f32, tag="seg_x")
                        gather(xt[:], src, gidx_s[:, j:j + 1], bound)
                        sm = wpool.tile([128, 128], f32, tag="sel")
                        nc.vector.tensor_tensor(
                            out=sm[:],
                            in0=slot_s[:, j:j + 1].to_broadcast([128, 128]),
                            in1=iota_f[:], op=mybir.AluOpType.is_equal)
                        nc.tensor.matmul(out=ps[:], lhsT=sm[:], rhs=xt[:],
                                         start=(m == 0), stop=(m == Tm - 1))
                        if track_sum:
                            track(xt)
                    o = wpool.tile([128, F], f32, tag="seg_o")
                    nc.vector.tensor_scalar_mul(o[:], ps[:], recip_s[:, g:g + 1])
                    nc.scalar.dma_start(out=out_t[g * 128:(g + 1) * 128, :], in_=o[:])
                # singleton groups: output row = gathered member row * recip
                for g in range(S.G_mm, S.G):
                    j = S.n_mm_tiles + (g - S.G_mm)
                    xt = wpool.tile([128, F], f32, tag="seg_x")
                    gather(xt[:], src, gidx_s[:, j:j + 1], bound)
                    if track_sum:
                        track(xt)
                    o = wpool.tile([128, F], f32, tag="seg_o")
                    nc.vector.tensor_scalar_mul(o[:], xt[:], recip_s[:, g:g + 1])
                    nc.scalar.dma_start(out=out_t[g * 128:(g + 1) * 128, :], in_=o[:])

            side(sp, p_gidx_s, p_slot_s, p_recip_s, out_p, pxf, NP_FULL - 1, True)
            side(sd, d_gidx_s, d_slot_s, d_recip_s, out_d, dxf, EP_FULL - 1, False)

            ps_sm = psa.tile([1, 128], f32, tag="ps_sm")
            nc.tensor.matmul(out=ps_sm[:], lhsT=ones[:], rhs=acc_sml[:],
                             start=True, stop=True)
            stat_sml = apool.tile([1, 128], f32, tag="stat_sml")
            nc.vector.tensor_copy(out=stat_sml[:], in_=ps_sm[0:1, :])
            nc.sync.dma_start(out=out_st[1:2, :], in_=stat_sml[:])

    nc.compile()
    return nc


# --------------------------------------------------------------------------
# main entry
# --------------------------------------------------------------------------

def kernel(primal_x, dual_x, att, primal_edge_index):
    global _last_results
    primal_x = np.asarray(primal_x, dtype=np.float32)
    dual_x = np.asarray(dual_x, dtype=np.float32)
    att = np.asarray(att, dtype=np.float32)
    pei = np.asarray(primal_edge_index, dtype=np.int32)
    src = pei[0].astype(np.int64)
    dst = pei[1].astype(np.int64)

    # ---------------- host graph bookkeeping ----------------
    mask = _pool_mask(att)
    labels = _connected_components(src, dst, mask)
    uniq, cluster = np.unique(labels, return_inverse=True)
    cluster = cluster.astype(np.int64)
    C = len(uniq)
    counts = np.bincount(cluster, minlength=C).astype(np.float64)
    giant = int(np.argmax(counts))

    cu = cluster[src]
    cv = cluster[dst]
    valid = cu != cv
    va = np.minimum(cu, cv)[valid]
    vb = np.maximum(cu, cv)[valid]
    vidx = np.nonzero(valid)[0]
    upairs, dinv = np.unique(np.stack([va, vb], axis=1), axis=0, return_inverse=True)
    D = len(upairs)
    dcounts = np.bincount(dinv, minlength=D).astype(np.float64)

    p_items = np.array([c for c in range(C) if c != giant], dtype=np.int64)
    members_all = _group_members(cluster, C)
    p_members = [members_all[c] for c in p_items]
    d_items = np.arange(D, dtype=np.int64)
    d_groups = _group_members(dinv, D)
    d_members = [vidx[g] for g in d_groups]  # global edge ids

    # padded full arrays: rows [N, NP_FULL) are zeros -> safe pad gather target
    sp = _Side(p_items, p_members, N)
    sd = _Side(d_items, d_members, E)

    # ---------------- per-core input maps ----------------
    pxf = np.zeros((NP_FULL, F), dtype=np.float32)
    pxf[:N] = primal_x
    dxf = np.zeros((EP_FULL, F), dtype=np.float32)
    dxf[:E] = dual_x
    iota128 = np.broadcast_to(np.arange(128, dtype=np.float32), (128, 128)).copy()
    in_maps = []
    for k in range(W):
        pxs = np.zeros((NSP, F), dtype=np.float32)
        pxs[:NS] = primal_x[k * NS:(k + 1) * NS]
        cus = np.zeros(ESP, dtype=np.int32)
        cvs = np.zeros(ESP, dtype=np.int32)
        cus[:ES] = cu[k * ES:(k + 1) * ES]
        cvs[:ES] = cv[k * ES:(k + 1) * ES]
        cls = np.zeros(NSP, dtype=np.int32)
        cls[:NS] = cluster[k * NS:(k + 1) * NS]
        def slot_arr(s):
            a = s.slot[k]
            return _wrap128(a if a.size else np.zeros(128, np.float32))

        in_maps.append({
            "pxf": pxf, "dxf": dxf, "pxs": pxs,
            "p_gidx": _wrap128(sp.gidx[k]),
            "p_slot": slot_arr(sp),
            "d_gidx": _wrap128(sd.gidx[k]),
            "d_slot": slot_arr(sd),
            "p_recip": _wrap128(sp.recips(k, counts[p_items]).astype(np.float32)),
            "d_recip": _wrap128(sd.recips(k, dcounts).astype(np.float32)),
            "cu": _wrap128(cus), "cv": _wrap128(cvs), "cl": _wrap128(cls),
            "iota": iota128,
        })

    # ---------------- build + run ----------------
    nc = _build_program(sp, sd)
    res = run_bass_kernel_spmd(nc, in_maps, list(range(W)))
    _last_results = res

    # ---------------- host assembly ----------------
    new_primal_x = np.zeros((N, F), dtype=np.float32)
    new_dual_x = np.zeros((E, F), dtype=np.float32)
    tot = np.zeros(F, dtype=np.float64)
    sml = np.zeros(F, dtype=np.float64)
    for k in range(W):
        o = sp.owned[k]
        if len(o):
            new_primal_x[p_items[o]] = res.results[k]["out_p"][: len(o)]
        od = sd.owned[k]
        if len(od):
            new_dual_x[d_items[od]] = res.results[k]["out_d"][: len(od)]
        st = res.results[k]["out_st"]
        tot += st[0]
        sml += st[1]
    new_primal_x[giant] = ((tot - sml) / max(counts[giant], 1.0)).astype(np.float32)

    npei = np.empty((2, E), dtype=np.int32)
    for k in range(W):
        oe = res.results[k]["out_e"]  # [2, 128, ET] wrapped
        npei[0, k * ES:(k + 1) * ES] = oe[0].T.reshape(-1)[:ES]
        npei[1, k * ES:(k + 1) * ES] = oe[1].T.reshape(-1)[:ES]

    cl_out = np.empty(N, dtype=np.int32)
    for k in range(W):
        cl_out[k * NS:(k + 1) * NS] = res.results[k]["out_cl"].T.reshape(-1)[:NS]

    return new_primal_x, new_dual_x, npei, cl_out


# revision 26
# speedup vs baseline: 3.7658x; 1.0064x over previous
"""DualPrimalEdgePooling on 8 TRN2 NeuronCores.

Strategy (graph/data parallel, collective-free):
  - Host computes the O(E) int32 bookkeeping: top-k pool mask, connected
    components (union-find by min label), cluster compaction, dual-pair
    compaction, and the gather/packing tables that drive the device.
  - The 8-core SPMD Bass kernel does all O(N*F)/O(E*F) feature work with
    fully independent cores (collectives on this part measure ~70-110us of
    fixed barrier cost, so ownership is arranged to need none):
      * primal_x/dual_x are visible to every core; segment ownership is
        round-robin.  Each core indirect-DMA gathers the member rows of its
        owned segments, reduces them with one-hot (iota==slot) selection
        matmuls accumulated in PSUM, applies the mean via a per-partition
        reciprocal scale, and writes compact outputs.
      * the giant component's sum is (total - sum of small members): each
        core column-sum reduces one contiguous 1/8 shard of primal_x (DVE
        accumulate + one ones-matmul) and the colsum of its gathered small
        members, emitting a [2,128] partial; the final (tot-small)/count for
        that single output row is folded on the host during unsharding.
      * new_primal_edge_index is rebuilt on-device (is_equal + select),
        cluster passes through the device.
  - Host scatters the compact device outputs into the full-size (mostly
    zero) result tensors.
"""

import os
import sys

for _p in ("/opt/trn_rl_repo",):
    if _p not in sys.path and os.path.isdir(_p):
        sys.path.insert(0, _p)

import numpy as np

import concourse.bass as bass
import concourse.bacc as bacc
import concourse.mybir as mybir
from concourse.bass_utils import run_bass_kernel_spmd
from concourse.tile import TileContext

N = 200_000
E = 600_000
F = 128
NUM_KEEP = 300_000
W = 8                     # cores
NS = N // W               # primal rows per shard  (25000)
ES = E // W               # dual rows / edges per shard (75000)
NSP = ((NS + 127) // 128) * 128   # padded shard rows (25088)
ESP = ((ES + 127) // 128) * 128   # padded shard rows (75008)
NP_FULL = NSP * W                 # padded full primal rows (200704)
EP_FULL = ESP * W                 # padded full dual rows (600064)
GATHER_BATCH = 1                  # offset columns per indirect DMA (multi-column
                                  # offsets gather garbage — verified on HW)

_last_results = None      # BassKernelResults of the most recent run (for test harness)


# --------------------------------------------------------------------------
# host-side graph bookkeeping
# --------------------------------------------------------------------------

def _pool_mask(att):
    order = np.argsort(-att, kind="stable")
    m = np.zeros(E, dtype=bool)
    m[order[: E - NUM_KEEP]] = True
    return m


def _connected_components(src, dst, mask):
    """labels[i] = min node id in i's component over masked edges."""
    parent = np.arange(N, dtype=np.int64)

    def find(a):
        while parent[a] != a:
            parent[a] = parent[parent[a]]
            a = parent[a]
        return a

    for a, b in zip(src[mask], dst[mask]):
        ra, rb = find(a), find(b)
        if ra != rb:
            if ra < rb:
                parent[rb] = ra
            else:
                parent[ra] = rb
    lab = np.empty(N, dtype=np.int64)
    for i in range(N):
        lab[i] = find(i)
    return lab


def _group_members(keys, num_keys):
    """For int array keys (>=0), return list-of-arrays members per key value."""
    order = np.argsort(keys, kind="stable")
    sk = keys[order]
    starts = np.searchsorted(sk, np.arange(num_keys), side="left")
    ends = np.searchsorted(sk, np.arange(num_keys), side="right")
    return [order[s:e] for s, e in zip(starts, ends)]


class _Side:
    """Packing tables for one segment-sum side (primal clusters / dual segs).

    Items are assigned round-robin to cores. Within a core, multi-member
    items come first: groups [0, G_mm) reduce via one-hot selection matmuls
    with exact per-group tile counts Tm_list[g]; groups [G_mm, G) hold only
    singletons, whose output row IS the gathered member row (no matmul).
    All group/tile counts are maxima across cores so the SPMD program is
    uniform; padding gathers a known zero row."""

    def __init__(self, item_ids, members_per_item, pad_row):
        self.item_ids = item_ids
        M = len(item_ids)
        owner = np.arange(M) % W
        sizes = np.array([len(members_per_item[i]) for i in range(M)], dtype=np.int64)
        self.owned = []
        n_multi = []
        for k in range(W):
            o = np.nonzero(owner == k)[0]
            multi = o[sizes[o] > 1]
            # size-ascending keeps per-group member counts similar across
            # cores, so the cross-core max tile count per group stays tight
            multi = multi[np.argsort(sizes[multi], kind="stable")]
            single = o[sizes[o] <= 1]
            self.owned.append(np.concatenate([multi, single]))
            n_multi.append(len(multi))
        self.G = max(1, max((len(o) + 127) // 128 for o in self.owned))
        self.G_mm = max((nm + 127) // 128 for nm in n_multi)
        if self.G_mm == 0 and max(n_multi) == 0:
            self.G_mm = 0
        # exact per-matmul-group tile counts (max across cores)
        rows_kg = np.zeros((W, max(self.G_mm, 1)), dtype=np.int64)
        for k in range(W):
            reg = self.owned[k][: self.G_mm * 128]
            for gi, ii in enumerate(reg):
                rows_kg[k, gi // 128] += sizes[ii]
        self.Tm_list = [max(1, int(np.max((rows_kg[:, g] + 127) // 128)))
                        for g in range(self.G_mm)]
        self.n_mm_tiles = sum(self.Tm_list)
        ncols = self.n_mm_tiles + (self.G - self.G_mm)
        self.ncols = ncols
        self.gidx = []
        self.slot = []
        tile_base = np.cumsum([0] + self.Tm_list)  # start tile of mm group g
        for k in range(W):
            gi = np.full(ncols * 128, pad_row, dtype=np.int64)
            sl = np.zeros(self.n_mm_tiles * 128, dtype=np.float32)
            fill = np.zeros(max(self.G_mm, 1), dtype=np.int64)
            for gidx_i, ii in enumerate(self.owned[k]):
                g, slot = divmod(gidx_i, 128)
                if g < self.G_mm:
                    for gid in members_per_item[ii]:
                        pos = tile_base[g] * 128 + fill[g]
                        gi[pos] = gid
                        sl[pos] = slot
                        fill[g] += 1
                else:
                    pos = (self.n_mm_tiles + (g - self.G_mm)) * 128 + slot
                    gi[pos] = members_per_item[ii][0]
            self.gidx.append(gi.astype(np.int32))
            self.slot.append(sl)

    def recips(self, k, counts_per_item):
        r = np.ones(self.G * 128, dtype=np.float32)
        o = self.owned[k]
        r[: len(o)] = 1.0 / np.maximum(counts_per_item[o], 1.0)
        return r


def _wrap128(a):
    """[T*128] -> [128, T] transposed tile layout (column t = tile t)."""
    assert a.size % 128 == 0
    return np.ascontiguousarray(a.reshape(-1, 128).T)


# --------------------------------------------------------------------------
# device program
# --------------------------------------------------------------------------

def _build_program(sp, sd):
    f32, i32 = mybir.dt.float32, mybir.dt.int32
    nc = bacc.Bacc(None, target_bir_lowering=False)

    Gp = sp.G
    Gd = sd.G
    pxf = nc.declare_dram_parameter("pxf", [NP_FULL, F], f32, isOutput=False)
    dxf = nc.declare_dram_parameter("dxf", [EP_FULL, F], f32, isOutput=False)
    pxs = nc.declare_dram_parameter("pxs", [NSP, F], f32, isOutput=False)
    p_gidx = nc.declare_dram_parameter("p_gidx", [128, sp.ncols], i32, isOutput=False)
    p_slot = nc.declare_dram_parameter("p_slot", [128, max(sp.n_mm_tiles, 1)], f32,
                                       isOutput=False)
    d_gidx = nc.declare_dram_parameter("d_gidx", [128, sd.ncols], i32, isOutput=False)
    d_slot = nc.declare_dram_parameter("d_slot", [128, max(sd.n_mm_tiles, 1)], f32,
                                       isOutput=False)
    p_recip = nc.declare_dram_parameter("p_recip", [128, Gp], f32, isOutput=False)
    d_recip = nc.declare_dram_parameter("d_recip", [128, Gd], f32, isOutput=False)
    cu_in = nc.declare_dram_parameter("cu", [128, ESP // 128], i32, isOutput=False)
    cv_in = nc.declare_dram_parameter("cv", [128, ESP // 128], i32, isOutput=False)
    cl_in = nc.declare_dram_parameter("cl", [128, NSP // 128], i32, isOutput=False)
    iota_in = nc.declare_dram_parameter("iota", [128, 128], f32, isOutput=False)

    out_p = nc.declare_dram_parameter("out_p", [Gp * 128, F], f32, isOutput=True)
    out_d = nc.declare_dram_parameter("out_d", [Gd * 128, F], f32, isOutput=True)
    out_e = nc.declare_dram_parameter("out_e", [2, 128, ESP // 128], i32, isOutput=True)
    out_cl = nc.declare_dram_parameter("out_cl", [128, NSP // 128], i32, isOutput=True)
    out_st = nc.declare_dram_parameter("out_st", [2, F], f32, isOutput=True)

    ET = ESP // 128
    NT = NSP // 128

    with TileContext(nc) as tc:
        with (
            tc.tile_pool(name="const", bufs=1) as cpool,
            tc.tile_pool(name="work", bufs=12) as wpool,
            tc.tile_pool(name="aux", bufs=1) as apool,
            tc.tile_pool(name="psa", bufs=1, space="PSUM") as psa,
            tc.tile_pool(name="psg", bufs=4, space="PSUM") as psg,
        ):
            # ---- constants / preloaded tables ----
            ones = cpool.tile([128, 1], f32, tag="ones")
            nc.vector.memset(ones[:], 1.0)
            iota_f = cpool.tile([128, 128], f32, tag="iota")
            nc.sync.dma_start(out=iota_f[:], in_=iota_in[:, :])

            def preload(t, tag):
                s = cpool.tile(list(t.shape), t.dtype, tag=tag)
                nc.sync.dma_start(out=s[:], in_=t[:, :])
                return s

            p_gidx_s = preload(p_gidx, "p_gidx")
            p_slot_s = preload(p_slot, "p_slot")
            d_gidx_s = preload(d_gidx, "d_gidx")
            d_slot_s = preload(d_slot, "d_slot")
            p_recip_s = preload(p_recip, "p_recip")
            d_recip_s = preload(d_recip, "d_recip")

            # ---- edge index rebuild + cluster passthrough ----
            cu_t = apool.tile([128, ET], i32, tag="cu")
            cv_t = apool.tile([128, ET], i32, tag="cv")
            nc.sync.dma_start(out=cu_t[:], in_=cu_in[:, :])
            nc.sync.dma_start(out=cv_t[:], in_=cv_in[:, :])
            eq = apool.tile([128, ET], i32, tag="eq")
            nc.vector.tensor_tensor(out=eq[:], in0=cu_t[:], in1=cv_t[:],
                                    op=mybir.AluOpType.is_equal)
            neg1 = apool.tile([128, ET], i32, tag="neg1")
            nc.vector.memset(neg1[:], -1)
            r0 = apool.tile([128, ET], i32, tag="r0")
            nc.vector.select(out=r0[:], mask=eq[:], on_true=neg1[:], on_false=cu_t[:])
            nc.sync.dma_start(out=out_e[0, :, :], in_=r0[:])
            r1 = apool.tile([128, ET], i32, tag="r1")
            nc.vector.select(out=r1[:], mask=eq[:], on_true=neg1[:], on_false=cv_t[:])
            nc.sync.dma_start(out=out_e[1, :, :], in_=r1[:])

            cl_t = apool.tile([128, NT], i32, tag="cl")
            nc.sync.dma_start(out=cl_t[:], in_=cl_in[:, :])
            nc.sync.dma_start(out=out_cl[:, :], in_=cl_t[:])

            # ---- shard column-sum (for giant cluster), DVE accumulate ----
            pxv = pxs[:, :].rearrange("(b t p) f -> b p t f", p=128, t=4)
            nb = pxv.shape[0]
            acc = cpool.tile([128, 512], f32, tag="acc")
            for b in range(nb):
                xt = wpool.tile([128, 512], f32, tag="cs_x")
                nc.sync.dma_start(out=xt[:], in_=pxv[b, :, :, :])
                if b == 0:
                    nc.vector.tensor_copy(out=acc[:], in_=xt[:])
                else:
                    nc.vector.tensor_add(out=acc[:], in0=acc[:], in1=xt[:])
            ps_cs = psa.tile([1, 512], f32, tag="ps_cs")
            nc.tensor.matmul(out=ps_cs[:], lhsT=ones[:], rhs=acc[:],
                             start=True, stop=True)
            stat_tot = apool.tile([1, 128], f32, tag="stat_tot")
            nc.vector.tensor_copy(out=stat_tot[:], in_=ps_cs[0:1, 0:128])
            for blk in range(1, 4):
                nc.vector.tensor_add(out=stat_tot[:], in0=stat_tot[:],
                                     in1=ps_cs[0:1, blk * 128:(blk + 1) * 128])
            nc.sync.dma_start(out=out_st[0:1, :], in_=stat_tot[:])

            # ---- segment-sum groups ----
            def gather(xt_ap, src, idx_col_ap, bound):
                # no bounds_check: every index (incl. padding) targets a real
                # row of the zero-padded tables, and skipping the check saves
                # a register write + per-descriptor compare on the Q7 path
                nc.gpsimd.indirect_dma_start(
                    out=xt_ap, out_offset=None, in_=src[:, :],
                    in_offset=bass.IndirectOffsetOnAxis(ap=idx_col_ap, axis=0))

            # small-member colsum accumulator (primal side only), DVE chain
            acc_sml = cpool.tile([128, 128], f32, tag="acc_sml")
            sml_state = {"n": 0}

            def track(xt):
                if sml_state["n"] == 0:
                    nc.vector.tensor_copy(out=acc_sml[:], in_=xt[:])
                else:
                    nc.vector.tensor_add(out=acc_sml[:], in0=acc_sml[:], in1=xt[:])
                sml_state["n"] += 1

            def side(S, gidx_s, slot_s, recip_s, out_t, src, bound, track_sum):
                tile_base = [0]
                for t in S.Tm_list:
                    tile_base.append(tile_base[-1] + t)
                # singleton groups first: their gathers have no PE dependency,
                # so the matmul groups' PE work overlaps the kernel tail
                for g in range(S.G_mm, S.G):
                    j = S.n_mm_tiles + (g - S.G_mm)
                    xt = wpool.tile([128, F], f32, tag="seg_x")
                    gather(xt[:], src, gidx_s[:, j:j + 1], bound)
                    if track_sum:
                        track(xt)
                    o = wpool.tile([128, F], f32, tag="seg_o")
                    nc.vector.tensor_scalar_mul(o[:], xt[:], recip_s[:, g:g + 1])
                    nc.scalar.dma_start(out=out_t[g * 128:(g + 1) * 128, :], in_=o[:])
                # matmul groups
                for g in range(S.G_mm):
                    ps = psg.tile([128, F], f32, tag="ps_g")
                    Tm = S.Tm_list[g]
                    for m in range(Tm):
                        j = tile_base[g] + m
                        xt = wpool.tile([128, F], f32, tag="seg_x")
                        gather(xt[:], src, gidx_s[:, j:j + 1], bound)
                        sm = wpool.tile([128, 128], f32, tag="sel")
                        nc.vector.tensor_tensor(
                            out=sm[:],
                            in0=slot_s[:, j:j + 1].to_broadcast([128, 128]),
                            in1=iota_f[:], op=mybir.AluOpType.is_equal)
                        nc.tensor.matmul(out=ps[:], lhsT=sm[:], rhs=xt[:],
                                         start=(m == 0), stop=(m == Tm - 1))
                        if track_sum:
                            track(xt)
                    o = wpool.tile([128, F], f32, tag="seg_o")
                    nc.vector.tensor_scalar_mul(o[:], ps[:], recip_s[:, g:g + 1])
                    nc.scalar.dma_start(out=out_t[g * 128:(g + 1) * 128, :], in_=o[:])

            side(sp, p_gidx_s, p_slot_s, p_recip_s, out_p, pxf, NP_FULL - 1, True)
            side(sd, d_gidx_s, d_slot_s, d_recip_s, out_d, dxf, EP_FULL - 1, False)

            ps_sm = psa.tile([1, 128], f32, tag="ps_sm")
            nc.tensor.matmul(out=ps_sm[:], lhsT=ones[:], rhs=acc_sml[:],
                             start=True, stop=True)
            stat_sml = apool.tile([1, 128], f32, tag="stat_sml")
            nc.vector.tensor_copy(out=stat_sml[:], in_=ps_sm[0:1, :])
            nc.sync.dma_start(out=out_st[1:2, :], in_=stat_sml[:])

    nc.compile()
    return nc


# --------------------------------------------------------------------------
# main entry
# --------------------------------------------------------------------------

def kernel(primal_x, dual_x, att, primal_edge_index):
    global _last_results
    primal_x = np.asarray(primal_x, dtype=np.float32)
    dual_x = np.asarray(dual_x, dtype=np.float32)
    att = np.asarray(att, dtype=np.float32)
    pei = np.asarray(primal_edge_index, dtype=np.int32)
    src = pei[0].astype(np.int64)
    dst = pei[1].astype(np.int64)

    # ---------------- host graph bookkeeping ----------------
    mask = _pool_mask(att)
    labels = _connected_components(src, dst, mask)
    uniq, cluster = np.unique(labels, return_inverse=True)
    cluster = cluster.astype(np.int64)
    C = len(uniq)
    counts = np.bincount(cluster, minlength=C).astype(np.float64)
    giant = int(np.argmax(counts))

    cu = cluster[src]
    cv = cluster[dst]
    valid = cu != cv
    va = np.minimum(cu, cv)[valid]
    vb = np.maximum(cu, cv)[valid]
    vidx = np.nonzero(valid)[0]
    upairs, dinv = np.unique(np.stack([va, vb], axis=1), axis=0, return_inverse=True)
    D = len(upairs)
    dcounts = np.bincount(dinv, minlength=D).astype(np.float64)

    p_items = np.array([c for c in range(C) if c != giant], dtype=np.int64)
    members_all = _group_members(cluster, C)
    p_members = [members_all[c] for c in p_items]
    d_items = np.arange(D, dtype=np.int64)
    d_groups = _group_members(dinv, D)
    d_members = [vidx[g] for g in d_groups]  # global edge ids

    # padded full arrays: rows [N, NP_FULL) are zeros -> safe pad gather target
    sp = _Side(p_items, p_members, N)
    sd = _Side(d_items, d_members, E)

    # ---------------- per-core input maps ----------------
    pxf = np.zeros((NP_FULL, F), dtype=np.float32)
    pxf[:N] = primal_x
    dxf = np.zeros((EP_FULL, F), dtype=np.float32)
    dxf[:E] = dual_x
    iota128 = np.broadcast_to(np.arange(128, dtype=np.float32), (128, 128)).copy()
    in_maps = []
    for k in range(W):
        pxs = np.zeros((NSP, F), dtype=np.float32)
        pxs[:NS] = primal_x[k * NS:(k + 1) * NS]
        cus = np.zeros(ESP, dtype=np.int32)
        cvs = np.zeros(ESP, dtype=np.int32)
        cus[:ES] = cu[k * ES:(k + 1) * ES]
        cvs[:ES] = cv[k * ES:(k + 1) * ES]
        cls = np.zeros(NSP, dtype=np.int32)
        cls[:NS] = cluster[k * NS:(k + 1) * NS]
        def slot_arr(s):
            a = s.slot[k]
            return _wrap128(a if a.size else np.zeros(128, np.float32))

        in_maps.append({
            "pxf": pxf, "dxf": dxf, "pxs": pxs,
            "p_gidx": _wrap128(sp.gidx[k]),
            "p_slot": slot_arr(sp),
            "d_gidx": _wrap128(sd.gidx[k]),
            "d_slot": slot_arr(sd),
            "p_recip": _wrap128(sp.recips(k, counts[p_items]).astype(np.float32)),
            "d_recip": _wrap128(sd.recips(k, dcounts).astype(np.float32)),
            "cu": _wrap128(cus), "cv": _wrap128(cvs), "cl": _wrap128(cls),
            "iota": iota128,
        })

    # ---------------- build + run ----------------
    nc = _build_program(sp, sd)
    res = run_bass_kernel_spmd(nc, in_maps, list(range(W)))
    _last_results = res

    # ---------------- host assembly ----------------
    new_primal_x = np.zeros((N, F), dtype=np.float32)
    new_dual_x = np.zeros((E, F), dtype=np.float32)
    tot = np.zeros(F, dtype=np.float64)
    sml = np.zeros(F, dtype=np.float64)
    for k in range(W):
        o = sp.owned[k]
        if len(o):
            new_primal_x[p_items[o]] = res.results[k]["out_p"][: len(o)]
        od = sd.owned[k]
        if len(od):
            new_dual_x[d_items[od]] = res.results[k]["out_d"][: len(od)]
        st = res.results[k]["out_st"]
        tot += st[0]
        sml += st[1]
    new_primal_x[giant] = ((tot - sml) / max(counts[giant], 1.0)).astype(np.float32)

    npei = np.empty((2, E), dtype=np.int32)
    for k in range(W):
        oe = res.results[k]["out_e"]  # [2, 128, ET] wrapped
        npei[0, k * ES:(k + 1) * ES] = oe[0].T.reshape(-1)[:ES]
        npei[1, k * ES:(k + 1) * ES] = oe[1].T.reshape(-1)[:ES]

    cl_out = np.empty(N, dtype=np.int32)
    for k in range(W):
        cl_out[k * NS:(k + 1) * NS] = res.results[k]["out_cl"].T.reshape(-1)[:NS]

    return new_primal_x, new_dual_x, npei, cl_out


# revision 29
# speedup vs baseline: 4.0518x; 1.0760x over previous
"""DualPrimalEdgePooling on 8 TRN2 NeuronCores.

Strategy (graph/data parallel, collective-free):
  - Host computes the O(E) int32 bookkeeping: top-k pool mask, connected
    components (union-find by min label), cluster compaction, dual-pair
    compaction, and the gather/packing tables that drive the device.
  - The 8-core SPMD Bass kernel does all O(N*F)/O(E*F) feature work with
    fully independent cores (collectives on this part measure ~70-110us of
    fixed barrier cost, so ownership is arranged to need none):
      * primal_x/dual_x are visible to every core; segment ownership is
        round-robin.  Each core indirect-DMA gathers the member rows of its
        owned segments, reduces them with one-hot (iota==slot) selection
        matmuls accumulated in PSUM, applies the mean via a per-partition
        reciprocal scale, and writes compact outputs.
      * the giant component's sum is (total - sum of small members): each
        core column-sum reduces one contiguous 1/8 shard of primal_x (DVE
        accumulate + one ones-matmul) and the colsum of its gathered small
        members, emitting a [2,128] partial; the final (tot-small)/count for
        that single output row is folded on the host during unsharding.
      * new_primal_edge_index is rebuilt on-device (is_equal + select),
        cluster passes through the device.
  - Host scatters the compact device outputs into the full-size (mostly
    zero) result tensors.
"""

import os
import sys

for _p in ("/opt/trn_rl_repo",):
    if _p not in sys.path and os.path.isdir(_p):
        sys.path.insert(0, _p)

import numpy as np

import concourse.bass as bass
import concourse.bacc as bacc
import concourse.mybir as mybir
from concourse.bass_utils import run_bass_kernel_spmd
from concourse.tile import TileContext

N = 200_000
E = 600_000
F = 128
NUM_KEEP = 300_000
W = 8                     # cores
NS = N // W               # primal rows per shard  (25000)
ES = E // W               # dual rows / edges per shard (75000)
NSP = ((NS + 127) // 128) * 128   # padded shard rows (25088)
ESP = ((ES + 127) // 128) * 128   # padded shard rows (75008)
NP_FULL = NSP * W                 # padded full primal rows (200704)
EP_FULL = ESP * W                 # padded full dual rows (600064)
GATHER_BATCH = 1                  # offset columns per indirect DMA (multi-column
                                  # offsets gather garbage — verified on HW)

_last_results = None      # BassKernelResults of the most recent run (for test harness)


# --------------------------------------------------------------------------
# host-side graph bookkeeping
# --------------------------------------------------------------------------

def _pool_mask(att):
    order = np.argsort(-att, kind="stable")
    m = np.zeros(E, dtype=bool)
    m[order[: E - NUM_KEEP]] = True
    return m


def _connected_components(src, dst, mask):
    """labels[i] = min node id in i's component over masked edges."""
    parent = np.arange(N, dtype=np.int64)

    def find(a):
        while parent[a] != a:
            parent[a] = parent[parent[a]]
            a = parent[a]
        return a

    for a, b in zip(src[mask], dst[mask]):
        ra, rb = find(a), find(b)
        if ra != rb:
            if ra < rb:
                parent[rb] = ra
            else:
                parent[ra] = rb
    lab = np.empty(N, dtype=np.int64)
    for i in range(N):
        lab[i] = find(i)
    return lab


def _group_members(keys, num_keys):
    """For int array keys (>=0), return list-of-arrays members per key value."""
    order = np.argsort(keys, kind="stable")
    sk = keys[order]
    starts = np.searchsorted(sk, np.arange(num_keys), side="left")
    ends = np.searchsorted(sk, np.arange(num_keys), side="right")
    return [order[s:e] for s, e in zip(starts, ends)]


class _Side:
    """Packing tables for one segment-sum side (primal clusters / dual segs).

    Items are assigned round-robin to cores. Within a core, multi-member
    items come first: groups [0, G_mm) reduce via one-hot selection matmuls
    with exact per-group tile counts Tm_list[g]; groups [G_mm, G) hold only
    singletons, whose output row IS the gathered member row (no matmul).
    All group/tile counts are maxima across cores so the SPMD program is
    uniform; padding gathers a known zero row."""

    def __init__(self, item_ids, members_per_item, pad_row):
        self.item_ids = item_ids
        M = len(item_ids)
        owner = np.arange(M) % W
        sizes = np.array([len(members_per_item[i]) for i in range(M)], dtype=np.int64)
        self.owned = []
        n_multi = []
        for k in range(W):
            o = np.nonzero(owner == k)[0]
            multi = o[sizes[o] > 1]
            # size-ascending keeps per-group member counts similar across
            # cores, so the cross-core max tile count per group stays tight
            multi = multi[np.argsort(sizes[multi], kind="stable")]
            single = o[sizes[o] <= 1]
            self.owned.append(np.concatenate([multi, single]))
            n_multi.append(len(multi))
        self.G = max(1, max((len(o) + 127) // 128 for o in self.owned))
        self.G_mm = max((nm + 127) // 128 for nm in n_multi)
        if self.G_mm == 0 and max(n_multi) == 0:
            self.G_mm = 0
        # exact per-matmul-group tile counts (max across cores)
        rows_kg = np.zeros((W, max(self.G_mm, 1)), dtype=np.int64)
        for k in range(W):
            reg = self.owned[k][: self.G_mm * 128]
            for gi, ii in enumerate(reg):
                rows_kg[k, gi // 128] += sizes[ii]
        self.Tm_list = [max(1, int(np.max((rows_kg[:, g] + 127) // 128)))
                        for g in range(self.G_mm)]
        self.n_mm_tiles = sum(self.Tm_list)
        ncols = self.n_mm_tiles + (self.G - self.G_mm)
        self.ncols = ncols
        self.gidx = []
        self.slot = []
        tile_base = np.cumsum([0] + self.Tm_list)  # start tile of mm group g
        for k in range(W):
            gi = np.full(ncols * 128, pad_row, dtype=np.int64)
            sl = np.zeros(self.n_mm_tiles * 128, dtype=np.float32)
            fill = np.zeros(max(self.G_mm, 1), dtype=np.int64)
            for gidx_i, ii in enumerate(self.owned[k]):
                g, slot = divmod(gidx_i, 128)
                if g < self.G_mm:
                    for gid in members_per_item[ii]:
                        pos = tile_base[g] * 128 + fill[g]
                        gi[pos] = gid
                        sl[pos] = slot
                        fill[g] += 1
                else:
                    pos = (self.n_mm_tiles + (g - self.G_mm)) * 128 + slot
                    gi[pos] = members_per_item[ii][0]
            self.gidx.append(gi.astype(np.int32))
            self.slot.append(sl)

    def recips(self, k, counts_per_item):
        r = np.ones(self.G * 128, dtype=np.float32)
        o = self.owned[k]
        r[: len(o)] = 1.0 / np.maximum(counts_per_item[o], 1.0)
        return r


def _wrap128(a):
    """[T*128] -> [128, T] transposed tile layout (column t = tile t)."""
    assert a.size % 128 == 0
    return np.ascontiguousarray(a.reshape(-1, 128).T)


# --------------------------------------------------------------------------
# device program
# --------------------------------------------------------------------------

def _build_program(sp, sd):
    f32, i32 = mybir.dt.float32, mybir.dt.int32
    nc = bacc.Bacc(None, target_bir_lowering=False)

    Gp = sp.G
    Gd = sd.G
    pxf = nc.declare_dram_parameter("pxf", [NP_FULL, F], f32, isOutput=False)
    dxf = nc.declare_dram_parameter("dxf", [EP_FULL, F], f32, isOutput=False)
    pxs = nc.declare_dram_parameter("pxs", [NSP, F], f32, isOutput=False)
    p_gidx = nc.declare_dram_parameter("p_gidx", [128, sp.ncols], i32, isOutput=False)
    p_slot = nc.declare_dram_parameter("p_slot", [128, max(sp.n_mm_tiles, 1)], f32,
                                       isOutput=False)
    d_gidx = nc.declare_dram_parameter("d_gidx", [128, sd.ncols], i32, isOutput=False)
    d_slot = nc.declare_dram_parameter("d_slot", [128, max(sd.n_mm_tiles, 1)], f32,
                                       isOutput=False)
    p_recip = nc.declare_dram_parameter("p_recip", [128, Gp], f32, isOutput=False)
    d_recip = nc.declare_dram_parameter("d_recip", [128, Gd], f32, isOutput=False)
    cu_in = nc.declare_dram_parameter("cu", [128, ESP // 128], i32, isOutput=False)
    cv_in = nc.declare_dram_parameter("cv", [128, ESP // 128], i32, isOutput=False)
    cl_in = nc.declare_dram_parameter("cl", [128, NSP // 128], i32, isOutput=False)
    iota_in = nc.declare_dram_parameter("iota", [128, 128], f32, isOutput=False)

    out_p = nc.declare_dram_parameter("out_p", [Gp * 128, F], f32, isOutput=True)
    out_d = nc.declare_dram_parameter("out_d", [Gd * 128, F], f32, isOutput=True)
    out_e = nc.declare_dram_parameter("out_e", [2, 128, ESP // 128], i32, isOutput=True)
    out_cl = nc.declare_dram_parameter("out_cl", [128, NSP // 128], i32, isOutput=True)
    out_st = nc.declare_dram_parameter("out_st", [2, F], f32, isOutput=True)

    ET = ESP // 128
    NT = NSP // 128

    with TileContext(nc) as tc:
        with (
            tc.tile_pool(name="const", bufs=1) as cpool,
            tc.tile_pool(name="work", bufs=16) as wpool,
            tc.tile_pool(name="aux", bufs=1) as apool,
            tc.tile_pool(name="psa", bufs=1, space="PSUM") as psa,
            tc.tile_pool(name="psg", bufs=5, space="PSUM") as psg,
        ):
            # ---- constants / preloaded tables ----
            ones = cpool.tile([128, 1], f32, tag="ones")
            nc.vector.memset(ones[:], 1.0)
            iota_f = cpool.tile([128, 128], f32, tag="iota")
            nc.sync.dma_start(out=iota_f[:], in_=iota_in[:, :])

            def preload(t, tag, eng=None):
                s = cpool.tile(list(t.shape), t.dtype, tag=tag)
                (eng or nc.sync).dma_start(out=s[:], in_=t[:, :])
                return s

            # gather indices load on gpsimd itself: the first indirect DMA
            # then has a same-engine dependency instead of waiting on the
            # sync ring's FIFO behind the big column-sum loads
            p_gidx_s = preload(p_gidx, "p_gidx", nc.gpsimd)
            d_gidx_s = preload(d_gidx, "d_gidx", nc.gpsimd)
            p_slot_s = preload(p_slot, "p_slot", nc.scalar)
            d_slot_s = preload(d_slot, "d_slot", nc.scalar)
            p_recip_s = preload(p_recip, "p_recip", nc.scalar)
            d_recip_s = preload(d_recip, "d_recip", nc.scalar)

            # ---- edge index rebuild + cluster passthrough ----
            cu_t = apool.tile([128, ET], i32, tag="cu")
            cv_t = apool.tile([128, ET], i32, tag="cv")
            nc.sync.dma_start(out=cu_t[:], in_=cu_in[:, :])
            nc.sync.dma_start(out=cv_t[:], in_=cv_in[:, :])
            eq = apool.tile([128, ET], i32, tag="eq")
            nc.vector.tensor_tensor(out=eq[:], in0=cu_t[:], in1=cv_t[:],
                                    op=mybir.AluOpType.is_equal)
            neg1 = apool.tile([128, ET], i32, tag="neg1")
            nc.vector.memset(neg1[:], -1)
            r0 = apool.tile([128, ET], i32, tag="r0")
            nc.vector.select(out=r0[:], mask=eq[:], on_true=neg1[:], on_false=cu_t[:])
            nc.sync.dma_start(out=out_e[0, :, :], in_=r0[:])
            r1 = apool.tile([128, ET], i32, tag="r1")
            nc.vector.select(out=r1[:], mask=eq[:], on_true=neg1[:], on_false=cv_t[:])
            nc.sync.dma_start(out=out_e[1, :, :], in_=r1[:])

            cl_t = apool.tile([128, NT], i32, tag="cl")
            nc.sync.dma_start(out=cl_t[:], in_=cl_in[:, :])
            nc.sync.dma_start(out=out_cl[:, :], in_=cl_t[:])

            # ---- shard column-sum (for giant cluster), DVE accumulate ----
            pxv = pxs[:, :].rearrange("(b t p) f -> b p t f", p=128, t=4)
            nb = pxv.shape[0]
            acc = cpool.tile([128, 512], f32, tag="acc")
            for b in range(nb):
                xt = wpool.tile([128, 512], f32, tag="cs_x")
                nc.sync.dma_start(out=xt[:], in_=pxv[b, :, :, :])
                if b == 0:
                    nc.vector.tensor_copy(out=acc[:], in_=xt[:])
                else:
                    nc.vector.tensor_add(out=acc[:], in0=acc[:], in1=xt[:])
            ps_cs = psa.tile([1, 512], f32, tag="ps_cs")
            nc.tensor.matmul(out=ps_cs[:], lhsT=ones[:], rhs=acc[:],
                             start=True, stop=True)
            stat_tot = apool.tile([1, 128], f32, tag="stat_tot")
            nc.vector.tensor_copy(out=stat_tot[:], in_=ps_cs[0:1, 0:128])
            for blk in range(1, 4):
                nc.vector.tensor_add(out=stat_tot[:], in0=stat_tot[:],
                                     in1=ps_cs[0:1, blk * 128:(blk + 1) * 128])
            nc.sync.dma_start(out=out_st[0:1, :], in_=stat_tot[:])

            # ---- segment-sum groups ----
            def gather(xt_ap, src, idx_col_ap, bound):
                # no bounds_check: every index (incl. padding) targets a real
                # row of the zero-padded tables, and skipping the check saves
                # a register write + per-descriptor compare on the Q7 path
                nc.gpsimd.indirect_dma_start(
                    out=xt_ap, out_offset=None, in_=src[:, :],
                    in_offset=bass.IndirectOffsetOnAxis(ap=idx_col_ap, axis=0))

            # small-member colsum accumulator (primal side only), DVE chain
            acc_sml = cpool.tile([128, 128], f32, tag="acc_sml")
            sml_state = {"n": 0}

            def track(xt):
                if sml_state["n"] == 0:
                    nc.vector.tensor_copy(out=acc_sml[:], in_=xt[:])
                else:
                    nc.vector.tensor_add(out=acc_sml[:], in0=acc_sml[:], in1=xt[:])
                sml_state["n"] += 1

            def side(S, gidx_s, slot_s, recip_s, out_t, src, bound, track_sum):
                tile_base = [0]
                for t in S.Tm_list:
                    tile_base.append(tile_base[-1] + t)
                # singleton groups first: their gathers have no PE dependency,
                # so the matmul groups' PE work overlaps the kernel tail
                for g in range(S.G_mm, S.G):
                    j = S.n_mm_tiles + (g - S.G_mm)
                    xt = wpool.tile([128, F], f32, tag="seg_x")
                    gather(xt[:], src, gidx_s[:, j:j + 1], bound)
                    if track_sum:
                        track(xt)
                    o = wpool.tile([128, F], f32, tag="seg_o")
                    nc.vector.tensor_scalar_mul(o[:], xt[:], recip_s[:, g:g + 1])
                    nc.scalar.dma_start(out=out_t[g * 128:(g + 1) * 128, :], in_=o[:])
                # matmul groups
                for g in range(S.G_mm):
                    ps = psg.tile([128, F], f32, tag="ps_g")
                    Tm = S.Tm_list[g]
                    for m in range(Tm):
                        j = tile_base[g] + m
                        xt = wpool.tile([128, F], f32, tag="seg_x")
                        gather(xt[:], src, gidx_s[:, j:j + 1], bound)
                        sm = wpool.tile([128, 128], f32, tag="sel")
                        nc.vector.tensor_tensor(
                            out=sm[:],
                            in0=slot_s[:, j:j + 1].to_broadcast([128, 128]),
                            in1=iota_f[:], op=mybir.AluOpType.is_equal)
                        nc.tensor.matmul(out=ps[:], lhsT=sm[:], rhs=xt[:],
                                         start=(m == 0), stop=(m == Tm - 1))
                        if track_sum:
                            track(xt)
                    o = wpool.tile([128, F], f32, tag="seg_o")
                    nc.vector.tensor_scalar_mul(o[:], ps[:], recip_s[:, g:g + 1])
                    nc.scalar.dma_start(out=out_t[g * 128:(g + 1) * 128, :], in_=o[:])

            side(sp, p_gidx_s, p_slot_s, p_recip_s, out_p, pxf, NP_FULL - 1, True)
            side(sd, d_gidx_s, d_slot_s, d_recip_s, out_d, dxf, EP_FULL - 1, False)

            ps_sm = psa.tile([1, 128], f32, tag="ps_sm")
            nc.tensor.matmul(out=ps_sm[:], lhsT=ones[:], rhs=acc_sml[:],
                             start=True, stop=True)
            stat_sml = apool.tile([1, 128], f32, tag="stat_sml")
            nc.vector.tensor_copy(out=stat_sml[:], in_=ps_sm[0:1, :])
            nc.sync.dma_start(out=out_st[1:2, :], in_=stat_sml[:])

    nc.compile()
    return nc


# --------------------------------------------------------------------------
# main entry
# --------------------------------------------------------------------------

def kernel(primal_x, dual_x, att, primal_edge_index):
    global _last_results
    primal_x = np.asarray(primal_x, dtype=np.float32)
    dual_x = np.asarray(dual_x, dtype=np.float32)
    att = np.asarray(att, dtype=np.float32)
    pei = np.asarray(primal_edge_index, dtype=np.int32)
    src = pei[0].astype(np.int64)
    dst = pei[1].astype(np.int64)

    # ---------------- host graph bookkeeping ----------------
    mask = _pool_mask(att)
    labels = _connected_components(src, dst, mask)
    uniq, cluster = np.unique(labels, return_inverse=True)
    cluster = cluster.astype(np.int64)
    C = len(uniq)
    counts = np.bincount(cluster, minlength=C).astype(np.float64)
    giant = int(np.argmax(counts))

    cu = cluster[src]
    cv = cluster[dst]
    valid = cu != cv
    va = np.minimum(cu, cv)[valid]
    vb = np.maximum(cu, cv)[valid]
    vidx = np.nonzero(valid)[0]
    upairs, dinv = np.unique(np.stack([va, vb], axis=1), axis=0, return_inverse=True)
    D = len(upairs)
    dcounts = np.bincount(dinv, minlength=D).astype(np.float64)

    p_items = np.array([c for c in range(C) if c != giant], dtype=np.int64)
    members_all = _group_members(cluster, C)
    p_members = [members_all[c] for c in p_items]
    d_items = np.arange(D, dtype=np.int64)
    d_groups = _group_members(dinv, D)
    d_members = [vidx[g] for g in d_groups]  # global edge ids

    # padded full arrays: rows [N, NP_FULL) are zeros -> safe pad gather target
    sp = _Side(p_items, p_members, N)
    sd = _Side(d_items, d_members, E)

    # ---------------- per-core input maps ----------------
    pxf = np.zeros((NP_FULL, F), dtype=np.float32)
    pxf[:N] = primal_x
    dxf = np.zeros((EP_FULL, F), dtype=np.float32)
    dxf[:E] = dual_x
    iota128 = np.broadcast_to(np.arange(128, dtype=np.float32), (128, 128)).copy()
    in_maps = []
    for k in range(W):
        pxs = np.zeros((NSP, F), dtype=np.float32)
        pxs[:NS] = primal_x[k * NS:(k + 1) * NS]
        cus = np.zeros(ESP, dtype=np.int32)
        cvs = np.zeros(ESP, dtype=np.int32)
        cus[:ES] = cu[k * ES:(k + 1) * ES]
        cvs[:ES] = cv[k * ES:(k + 1) * ES]
        cls = np.zeros(NSP, dtype=np.int32)
        cls[:NS] = cluster[k * NS:(k + 1) * NS]
        def slot_arr(s):
            a = s.slot[k]
            return _wrap128(a if a.size else np.zeros(128, np.float32))

        in_maps.append({
            "pxf": pxf, "dxf": dxf, "pxs": pxs,
            "p_gidx": _wrap128(sp.gidx[k]),
            "p_slot": slot_arr(sp),
            "d_gidx": _wrap128(sd.gidx[k]),
            "d_slot": slot_arr(sd),
            "p_recip": _wrap128(sp.recips(k, counts[p_items]).astype(np.float32)),
            "d_recip": _wrap128(sd.recips(k, dcounts).astype(np.float32)),
            "cu": _wrap128(cus), "cv": _wrap128(cvs), "cl": _wrap128(cls),
            "iota": iota128,
        })

    # ---------------- build + run ----------------
    nc = _build_program(sp, sd)
    res = run_bass_kernel_spmd(nc, in_maps, list(range(W)))
    _last_results = res

    # ---------------- host assembly ----------------
    new_primal_x = np.zeros((N, F), dtype=np.float32)
    new_dual_x = np.zeros((E, F), dtype=np.float32)
    tot = np.zeros(F, dtype=np.float64)
    sml = np.zeros(F, dtype=np.float64)
    for k in range(W):
        o = sp.owned[k]
        if len(o):
            new_primal_x[p_items[o]] = res.results[k]["out_p"][: len(o)]
        od = sd.owned[k]
        if len(od):
            new_dual_x[d_items[od]] = res.results[k]["out_d"][: len(od)]
        st = res.results[k]["out_st"]
        tot += st[0]
        sml += st[1]
    new_primal_x[giant] = ((tot - sml) / max(counts[giant], 1.0)).astype(np.float32)

    npei = np.empty((2, E), dtype=np.int32)
    for k in range(W):
        oe = res.results[k]["out_e"]  # [2, 128, ET] wrapped
        npei[0, k * ES:(k + 1) * ES] = oe[0].T.reshape(-1)[:ES]
        npei[1, k * ES:(k + 1) * ES] = oe[1].T.reshape(-1)[:ES]

    cl_out = np.empty(N, dtype=np.int32)
    for k in range(W):
        cl_out[k * NS:(k + 1) * NS] = res.results[k]["out_cl"].T.reshape(-1)[:NS]

    return new_primal_x, new_dual_x, npei, cl_out
